# revision 25
# baseline (speedup 1.0000x reference)
"""Trainium2 Bass kernel for AttentionFFNBlock (B=2, L=2048, D=1024, H=16, FF=4096).

Sharding (8 cores, zero cross-core communication):
  core c -> batch b = c//4, group slot g = c%4.
  Each core owns 512 query rows of its batch, interleaved in 128-row blocks
  for causal load balance: global row = (2p+s)*512 + g*128 + i for local row
  r = p*256 + s*128 + i.  The core computes K/V for the full sequence
  (replicated inside the batch group), attention for its rows over all 16
  heads, then out-proj + LN1 + FFN + LN2 for its rows only.  Causality is
  enforced with per-core additive masks passed as input data (SPMD-safe).

Schedule: x arrives pre-transposed from the host (no DMA transposes); Q and
the first K/V chunks are projected up front; the remaining K / V projections
are interleaved into the attention head-pair loop so the PE stays dense
while the ACT engine works through the exp()s.  Scores skip dead (fully
masked) column blocks; head pairs share one exp instruction and alternate
PE row-groups (tile_position) so K=64 matmuls pack the array.  Wo/W1 are
prefetched as soon as SBUF frees up.  fc2 runs in two passes (rc pairs) so
the LN2 epilogues overlap the second pass's matmuls.

All matmuls in bf16 (fp32 PSUM accumulation); norms/softmax in fp32.
"""

import numpy as np
import ml_dtypes

import concourse.bass as bass
import concourse.mybir as mybir
import concourse.tile as tile
from concourse import bacc
from concourse.bass_utils import run_bass_kernel_spmd
from concourse.masks import make_identity

F32 = mybir.dt.float32
BF16 = mybir.dt.bfloat16
AF = mybir.ActivationFunctionType
ALU = mybir.AluOpType

N_CORES = 8
B, L, D = 2, 2048, 1024
H, HD = 16, 64
DFF = 4096
EPS = 1e-5
P = 128
NEG = -1e9

IC = D // P        # 8 contraction chunks of the model dim
TC = L // P        # 16 token chunks
FC = DFF // P      # 32 ff chunks
NPAIR = 8          # head pairs (= oc chunks)

_CACHE = {}


def _build():
    nc = bacc.Bacc("TRN2", target_bir_lowering=False, debug=False,
                   num_devices=N_CORES)

    def din(name, shape, dt=F32):
        return nc.dram_tensor(name, shape, dt, kind="ExternalInput").ap()

    io = dict(
        xT=din("xT", [D, L], BF16),               # x[b]^T (K/V source)
        xrT=din("xrT", [D, 512], BF16),           # owned rows^T (Q source)
        xr=din("xr", [512, D], F32),              # owned rows (residual)
        wq=din("wq", [D, D], BF16), wk=din("wk", [D, D], BF16),
        wv=din("wv", [D, D], BF16), wo=din("wo", [D, D], BF16),
        w1=din("w1", [D, DFF], BF16), w2=din("w2", [DFF, D], BF16),
        bq=din("bq", [D]), bk=din("bk", [D]), bv=din("bv", [D], BF16),
        bo=din("bo", [D], BF16), b1=din("b1", [DFF]), b2=din("b2", [D], BF16),
        g1=din("g1", [D], BF16), be1=din("be1", [D], BF16),
        g2=din("g2", [D], BF16), be2=din("be2", [D], BF16),
        cmask=din("cmask", [4, P, P], BF16),
        out=nc.dram_tensor("out", [512, D], BF16, kind="ExternalOutput").ap(),
    )

    with tile.TileContext(nc) as tc:
        _emit(nc, tc, io)
    nc.compile()
    return nc


def _layernorm(nc, pool, acc, eps_t, g_t, b_t, out_ap, g_eng=None,
               b_eng=None):
    """LayerNorm over the free axis (D=1024) of acc [128, 1024] -> out_ap."""
    stats = pool.tile([P, 2, 6], F32, tag="ln_stats")
    for sg in range(2):
        nc.vector.bn_stats(out=stats[:, sg, :], in_=acc[:, sg * 512:(sg + 1) * 512])
    mv = pool.tile([P, 2], F32, tag="ln_mv")
    nc.vector.bn_aggr(out=mv[:], in_=stats[:])
    rstd = pool.tile([P, 1], F32, tag="ln_rstd")
    nc.scalar.activation(out=rstd[:], in_=mv[:, 1:2], func=AF.Sqrt,
                         bias=eps_t[:], scale=1.0)
    nc.vector.reciprocal(out=rstd[:], in_=rstd[:])
    nmr = pool.tile([P, 1], F32, tag="ln_nmr")
    nc.vector.tensor_scalar(out=nmr[:], in0=mv[:, 0:1], scalar1=rstd[:],
                            scalar2=-1.0, op0=ALU.mult, op1=ALU.mult)
    u = pool.tile([P, D], BF16, tag="ln_u")
    nc.scalar.activation(out=u[:], in_=acc[:], func=AF.Identity,
                         bias=nmr[:], scale=rstd[:])
    (g_eng or nc.gpsimd).tensor_tensor(out=u[:], in0=u[:], in1=g_t[:, :],
                                       op=ALU.mult)
    (b_eng or nc.vector).tensor_tensor(out=out_ap, in0=u[:], in1=b_t[:, :],
                                       op=ALU.add)


def _emit(nc, tc, io):
    out = io["out"]

    with tc.tile_pool(name="const", bufs=1) as const:
        ao_pool = tc.alloc_tile_pool(name="ao_pool", bufs=1, side="right")
        # ---- constants / biases (tiles now; DMAs deferred past wk/xT) ----
        bq_t = const.tile([P, IC], F32)
        bk_t = const.tile([P, IC], F32)
        b1_t = const.tile([P, FC], F32)
        row_vecs = {}
        for nm in ("bv", "bo", "b2", "g1", "be1", "g2", "be2"):
            dt = F32 if nm.startswith("nope") else BF16
            rv = const.tile([P, D], dt, name=f"cv_{nm}")
            row_vecs[nm] = rv
        bv_t, bo_t, b2_t = row_vecs["bv"], row_vecs["bo"], row_vecs["b2"]
        g1_t, be1_t = row_vecs["g1"], row_vecs["be1"]
        g2_t, be2_t = row_vecs["g2"], row_vecs["be2"]
        cm_t = const.tile([P, 4, P], BF16)
        eps_t = const.tile([P, 1], F32)
        ident = const.tile([P, P], BF16)

        def early_dmas():
            nc.sync.dma_start(bq_t[:], io["bq"].rearrange("(o p) -> p o", p=P))
            nc.sync.dma_start(bk_t[:], io["bk"].rearrange("(o p) -> p o", p=P))
            nc.sync.dma_start(b1_t[:], io["b1"].rearrange("(f p) -> p f", p=P))
            nc.sync.dma_start(row_vecs["bv"][:],
                              io["bv"][None, :].to_broadcast([P, D]))
            nc.vector.memset(eps_t[:], EPS)

        def const_dmas():
            nc.sync.dma_start(cm_t[:], io["cmask"].rearrange("i p q -> p i q"))
            for nm in ("bo", "b2", "g1", "be1", "g2", "be2"):
                nc.sync.dma_start(row_vecs[nm][:],
                                  io[nm][None, :].to_broadcast([P, D]))
            make_identity(nc, ident[:])

        aoT = ao_pool.tile([P, IC, 512], BF16)   # attention output^T

        kv_pool = tc.alloc_tile_pool(name="kv_pool", bufs=1)
        ptile = tc.alloc_tile_pool(name="ptile", bufs=3)
        rtile = tc.alloc_tile_pool(name="rtile", bufs=2)
        spsum = tc.alloc_tile_pool(name="spsum", bufs=2, space="PSUM")
        avpsum = tc.alloc_tile_pool(name="avpsum", bufs=1, space="PSUM")
        if True:
            kT = kv_pool.tile([P, IC, L], BF16)
            v_all = kv_pool.tile([P, TC, H, HD + 1], BF16)
            qT = kv_pool.tile([P, IC, 512], BF16)
            nc.vector.memset(v_all[:, :, :, HD:], 1.0)

            proj_stream = []   # deferred (emit_mms, epilogue) generators

            def drain_proj(n):
                """Emit up to n deferred projection matmuls."""
                while n > 0 and proj_stream:
                    gen = proj_stream[0]
                    try:
                        next(gen)
                        n -= 1
                    except StopIteration:
                        proj_stream.pop(0)

            def attention(pair):
                oc = pair
                hA, hB = 2 * pair, 2 * pair + 1
                pavA = avpsum.tile([HD + 1, 512], F32, tag="avA")
                pavB = avpsum.tile([HD + 1, 512], F32, tag="avB")
                pts = []
                for kc in range(TC):
                    j0 = kc // 4
                    n0 = j0 * P
                    ps = spsum.tile([P, 2, 512], F32, tag="s")
                    nc.tensor.matmul(
                        ps[:, 0, n0:512],
                        kT[0:HD, oc, kc * P:(kc + 1) * P],
                        qT[0:HD, oc, n0:512], start=True, stop=True)
                    nc.tensor.matmul(
                        ps[:, 1, n0:512],
                        kT[HD:P, oc, kc * P:(kc + 1) * P],
                        qT[HD:P, oc, n0:512], start=True, stop=True)
                    pt = ptile.tile([P, 2, 512], BF16, tag="p")
                    nc.scalar.activation(out=pt[:, :, n0:512],
                                         in_=ps[:, :, n0:512],
                                         func=AF.Exp, scale=0.125)
                    # diagonal-window causal mask on block j0 (both heads):
                    # multiply by 0/1 post-exp (gpsimd cannot touch PSUM)
                    for j in range(2):
                        nc.gpsimd.tensor_tensor(
                            out=pt[:, j, n0:n0 + P], in0=pt[:, j, n0:n0 + P],
                            in1=cm_t[:, kc % 4, :], op=ALU.mult)
                    pts.append((kc, n0, pt))
                    drain_proj(4 if pair < 3 else 2)
                    # AV for the previous chunk (pipelined one deep)
                    if len(pts) >= 2:
                        pkc, pn0, ppt = pts.pop(0)
                        for j, (h, pav) in enumerate(((hA, pavA), (hB, pavB))):
                            nc.tensor.matmul(
                                pav[:, pn0:512], v_all[:, pkc, h, :],
                                ppt[:, j, pn0:512], start=(pkc == 0),
                                stop=False, skip_group_check=True)
                pkc, pn0, ppt = pts.pop(0)
                for j, (h, pav) in enumerate(((hA, pavA), (hB, pavB))):
                    nc.tensor.matmul(
                        pav[:, pn0:512], v_all[:, pkc, h, :],
                        ppt[:, j, pn0:512], start=False, stop=True,
                        skip_group_check=True)
                for hp, pav in ((0, pavA), (HD, pavB)):
                    rec = rtile.tile([1, 512], F32, tag="rec")
                    nc.vector.reciprocal(rec[:], pav[HD:HD + 1, :])
                    rec_b = rtile.tile([HD, 512], F32, tag="rec_b")
                    nc.gpsimd.partition_broadcast(rec_b[:], rec[0:1, :])
                    nc.vector.tensor_tensor(
                        out=aoT[hp:hp + HD, oc, :],
                        in0=pav[:HD, :], in1=rec_b[:], op=ALU.mult)

            # ---- projections (pairs 0..5 overlap with x_pool live) ----
            with (
                tc.tile_pool(name="x_pool", bufs=1) as x_pool,
                tc.tile_pool(name="ppsum", bufs=2, space="PSUM") as ppsum,
            ):
                wk_t = x_pool.tile([P, IC, D], BF16)
                xT_t = x_pool.tile([P, IC, L], BF16)
                wq_t = x_pool.tile([P, IC, D], BF16)
                xrT_t = x_pool.tile([P, IC, 512], BF16)
                wv_t = x_pool.tile([P, IC, D], BF16)
                wkr = io["wk"].rearrange("(i p) n -> p i n", p=P)
                wqr = io["wq"].rearrange("(i p) n -> p i n", p=P)
                wvr = io["wv"].rearrange("(i p) n -> p i n", p=P)
                xTr = io["xT"].rearrange("(i p) n -> p i n", p=P)
                early_dmas()
                nc.sync.dma_start(wk_t[:, :, 0:512], wkr[:, :, 0:512])
                nc.sync.dma_start(xT_t[:, :, 0:512], xTr[:, :, 0:512])
                nc.sync.dma_start(wq_t[:, :, 0:512], wqr[:, :, 0:512])
                nc.sync.dma_start(xrT_t[:],
                                  io["xrT"].rearrange("(i p) n -> p i n", p=P))
                nc.sync.dma_start(wv_t[:, :, 0:512], wvr[:, :, 0:512])
                nc.sync.dma_start(xT_t[:, :, 512:1024], xTr[:, :, 512:1024])
                nc.sync.dma_start(xT_t[:, :, 1024:1536], xTr[:, :, 1024:1536])
                nc.sync.dma_start(xT_t[:, :, 1536:2048], xTr[:, :, 1536:2048])
                nc.sync.dma_start(wk_t[:, :, 512:1024], wkr[:, :, 512:1024])
                nc.sync.dma_start(wq_t[:, :, 512:1024], wqr[:, :, 512:1024])
                nc.sync.dma_start(wv_t[:, :, 512:1024], wvr[:, :, 512:1024])
                const_dmas()

                def k_proj(oc):
                    for tcc in range(4):
                        ps = ppsum.tile([P, 512], F32, tag="proj")
                        for ic in range(IC):
                            nc.tensor.matmul(
                                ps[:], wk_t[:, ic, oc * P:(oc + 1) * P],
                                xT_t[:, ic, tcc * 512:(tcc + 1) * 512],
                                start=(ic == 0), stop=(ic == IC - 1))
                            yield
                        nc.vector.tensor_scalar_add(
                            out=kT[:, oc, tcc * 512:(tcc + 1) * 512],
                            in0=ps[:], scalar1=bk_t[:, oc:oc + 1])

                def q_proj(oc):
                    ps = ppsum.tile([P, 512], F32, tag="proj")
                    for ic in range(IC):
                        nc.tensor.matmul(
                            ps[:], wq_t[:, ic, oc * P:(oc + 1) * P],
                            xrT_t[:, ic, :],
                            start=(ic == 0), stop=(ic == IC - 1))
                        yield
                    nc.vector.tensor_scalar_add(
                        out=qT[:, oc, :], in0=ps[:], scalar1=bq_t[:, oc:oc + 1])

                def v_proj(tcc, hf):
                    ps = ppsum.tile([P, 512], F32, tag="proj")
                    for ic in range(IC):
                        nc.tensor.matmul(
                            ps[:], xT_t[:, ic, tcc * P:(tcc + 1) * P],
                            wv_t[:, ic, hf * 512:(hf + 1) * 512],
                            start=(ic == 0), stop=(ic == IC - 1))
                        yield
                    nc.vector.tensor_tensor(
                        out=v_all[:, tcc, hf * 8:(hf + 1) * 8, :HD],
                        in0=ps.rearrange("p (h d) -> p h d", d=HD),
                        in1=bv_t[:, hf * 512:(hf + 1) * 512]
                        .rearrange("p (h d) -> p h d", d=HD),
                        op=ALU.add)

                # upfront, ordered to match serial DMA arrival
                def adv(gen, n):
                    for _ in range(n):
                        try:
                            next(gen)
                        except StopIteration:
                            return
                k0, k1 = k_proj(0), k_proj(1)
                qs = [q_proj(oc) for oc in range(IC)]
                v0s = [v_proj(tcc, 0) for tcc in range(TC)]
                adv(k0, 8)                       # K0.tcc0 (wk0+xT0)
                for oc in range(4):
                    adv(qs[oc], 9)               # Q0-3 (wq0+xrT)
                for tcc in range(4):
                    adv(v0s[tcc], 9)             # V0 tcc0-3 (wv0+xT0)
                adv(k0, 100)                     # K0 rest (xT1-3)
                adv(k1, 32)                      # K1 (wk1)
                for oc in range(4, IC):
                    adv(qs[oc], 9)               # Q4-7 (wq1)
                for tcc in range(4, TC):
                    adv(v0s[tcc], 9)             # V0 rest
                for g in [k0, k1] + qs + v0s:
                    adv(g, 100)
                # deferred: K2,K3, all of V1, K4..K7 — drained inside attention
                proj_stream.extend([k_proj(2), k_proj(3)])
                proj_stream.extend(v_proj(tcc, 1) for tcc in range(TC))
                proj_stream.extend(k_proj(oc) for oc in range(4, IC))

                for pair in range(7):
                    attention(pair)
                drain_proj(1 << 30)

            # x_pool freed: prefetch xr + wo under attn 7 (right side)
            xrr_pool = tc.alloc_tile_pool(name="xrr_pool", bufs=1, side="right")
            xr_nat = xrr_pool.tile([P, 4, D], F32)
            nc.sync.dma_start(xr_nat[:],
                              io["xr"].rearrange("(rc p) d -> p rc d", p=P))
            wo_pool = tc.alloc_tile_pool(name="wo_pool", bufs=1, side="right")
            wo_t = wo_pool.tile([P, IC, D], BF16)
            wor = io["wo"].rearrange("(i p) n -> p i n", p=P)
            for h2 in range(2):
                nc.sync.dma_start(wo_t[:, :, h2 * 512:(h2 + 1) * 512],
                                  wor[:, :, h2 * 512:(h2 + 1) * 512])

            attention(7)

            # free the attention pools (non-LIFO: wo/w1a stay live)
            avpsum.release()
            spsum.release()
            rtile.release()
            ptile.release()
            kv_pool.release()

            w1_pool = tc.alloc_tile_pool(name="w1_pool", bufs=1)
            w1_t = w1_pool.tile([P, IC, DFF], BF16)
            w1r = io["w1"].rearrange("(i p) n -> p i n", p=P)
            for c in range(8):
                nc.sync.dma_start(w1_t[:, :, c * 512:(c + 1) * 512],
                                  w1r[:, :, c * 512:(c + 1) * 512])


            if True:
                # ---- out-proj + LN1 + transpose ----
                with tc.tile_pool(name="t_pool", bufs=1) as t_pool:
                    tbf = t_pool.tile([P, 4, D], BF16)    # LN1 out (residual)
                    tT = t_pool.tile([P, IC, 512], BF16)  # LN1 out transposed

                    with (
                        tc.tile_pool(name="lnt", bufs=4) as lnt,
                        tc.tile_pool(name="opsum", bufs=4, space="PSUM") as opsum,
                        tc.tile_pool(name="trpsum", bufs=4, space="PSUM") as trpsum,
                    ):
                        for rc in range(4):
                            acc = lnt.tile([P, D], F32, tag="acc")
                            for n2 in range(2):
                                ps = opsum.tile([P, 512], F32, tag="o")
                                for dc in range(IC):
                                    nc.tensor.matmul(
                                        ps[:], aoT[:, dc, rc * P:(rc + 1) * P],
                                        wo_t[:, dc, n2 * 512:(n2 + 1) * 512],
                                        start=(dc == 0), stop=(dc == IC - 1))
                                nc.vector.tensor_tensor(
                                    out=acc[:, n2 * 512:(n2 + 1) * 512],
                                    in0=ps[:],
                                    in1=xr_nat[:, rc, n2 * 512:(n2 + 1) * 512],
                                    op=ALU.add)
                            nc.vector.tensor_tensor(
                                out=acc[:], in0=acc[:], in1=bo_t[:, :],
                                op=ALU.add)
                            _layernorm(nc, lnt, acc, eps_t, g1_t, be1_t,
                                       tbf[:, rc, :])
                        for rc in range(4):
                            for ic in range(IC):
                                pst = trpsum.tile([P, P], BF16, tag="tr")
                                nc.tensor.transpose(
                                    pst[:], tbf[:, rc, ic * P:(ic + 1) * P],
                                    ident[:])
                                if ic % 2 == 0:
                                    nc.vector.tensor_copy(
                                        tT[:, ic, rc * P:(rc + 1) * P], pst[:])
                                else:
                                    nc.scalar.copy(
                                        tT[:, ic, rc * P:(rc + 1) * P], pst[:])

                    wo_pool.release()
                    xrr_pool.release()
                    ao_pool.release()

                    # ================= FFN =================
                    w2_pool = tc.alloc_tile_pool(name="w2_pool", bufs=1)
                    w2_t = w2_pool.tile([P, FC, D], BF16)
                    w2r = io["w2"].rearrange("(f p) n -> p f n", p=P)
                    for grp in range(8):
                        nc.sync.dma_start(w2_t[:, grp * 4:(grp + 1) * 4, :],
                                          w2r[:, grp * 4:(grp + 1) * 4, :])
                    with (
                        tc.tile_pool(name="h_pool", bufs=1) as h_pool,
                        tc.tile_pool(name="fpsum", bufs=2, space="PSUM") as fpsum,
                        tc.tile_pool(name="ypsum", bufs=3, space="PSUM") as ypsum,
                    ):
                        hT = h_pool.tile([P, FC, 512], BF16)
                        psy = {}

                        def fc2_mms(fc, rcs):
                            for rc in rcs:
                                for n2 in range(2):
                                    nc.tensor.matmul(
                                        psy[rc][:, n2, :],
                                        hT[:, fc, rc * P:(rc + 1) * P],
                                        w2_t[:, fc, n2 * 512:(n2 + 1) * 512],
                                        start=(fc == 0), stop=(fc == FC - 1))

                        finbox = {}

                        def epilogue(rc):
                            fin = finbox["p"]
                            acc = fin.tile([P, D], F32, tag="acc2", bufs=2)
                            for n2 in range(2):
                                nc.vector.tensor_tensor(
                                    out=acc[:, n2 * 512:(n2 + 1) * 512],
                                    in0=psy[rc][:, n2, :],
                                    in1=tbf[:, rc, n2 * 512:(n2 + 1) * 512],
                                    op=ALU.add)
                            nc.vector.tensor_tensor(
                                out=acc[:], in0=acc[:], in1=b2_t[:, :],
                                op=ALU.add)
                            res = fin.tile([P, D], BF16, tag="res", bufs=2)
                            _layernorm(nc, fin, acc, eps_t, g2_t, be2_t,
                                       res[:], g_eng=nc.vector,
                                       b_eng=nc.gpsimd)
                            nc.sync.dma_start(
                                out.rearrange("(rc p) d -> p rc d", p=P)[:, rc, :],
                                res[:])

                        # pass 1: fc1 + fc2 for rc 0,1,2 interleaved per fc
                        psy[0] = ypsum.tile([P, 2, 512], F32, tag="y", name="psy0")
                        psy[1] = ypsum.tile([P, 2, 512], F32, tag="y", name="psy1")
                        psy[2] = ypsum.tile([P, 2, 512], F32, tag="y", name="psy2")
                        for grp in range(8):
                            for k in range(4):
                                fc = grp * 4 + k
                                ps = fpsum.tile([P, 512], F32, tag="f1")
                                for ic in range(IC):
                                    nc.tensor.matmul(
                                        ps[:],
                                        w1_t[:, ic, fc * P:(fc + 1) * P],
                                        tT[:, ic, :],
                                        start=(ic == 0), stop=(ic == IC - 1))
                                nc.scalar.activation(out=hT[:, fc, :], in_=ps[:],
                                                     func=AF.Gelu,
                                                     bias=b1_t[:, fc:fc + 1],
                                                     scale=1.0)
                                fc2_mms(fc, (0, 1, 2))
                        finbox["p"] = tc.alloc_tile_pool(name="fin", bufs=1)
                        epilogue(0)
                        epilogue(1)
                        epilogue(2)
                        # pass 2: fc2 for rc3 (w2 already prefetched)
                        psy[3] = ypsum.tile([P, 2, 512], F32, tag="y", name="psy3")
                        for fc in range(FC):
                            fc2_mms(fc, (3,))
                        epilogue(3)
                        finbox["p"].release()
                    w2_pool.release()

            w1_pool.release()


def _row_index(g):
    idx = np.empty(512, dtype=np.int64)
    r = 0
    for p in range(2):
        for s in range(2):
            j = 2 * p + s
            base = j * 512 + g * 128
            idx[r:r + 128] = np.arange(base, base + 128)
            r += 128
    return idx


def _causal_masks(g):
    kj = np.arange(P)[:, None]
    qi = np.arange(P)[None, :]
    m = np.empty((4, P, P), dtype=np.float32)
    for i in range(4):
        m[i] = np.where(kj <= qi + (g - i) * P, 1.0, 0.0)
    return m


def kernel(**inputs):
    if "nc" not in _CACHE:
        _CACHE["nc"] = _build()
    nc = _CACHE["nc"]

    bf = ml_dtypes.bfloat16
    x = np.asarray(inputs["x"], dtype=np.float32)
    w_bf = {k: np.ascontiguousarray(
        np.asarray(inputs[k], dtype=np.float32).astype(bf))
        for k in ("Wq", "Wk", "Wv", "Wo", "W1", "W2")}
    vecs = {k: np.ascontiguousarray(np.asarray(inputs[k], dtype=np.float32))
            for k in ("bq", "bk", "bv", "bo", "b1", "b2", "g1", "be1", "g2",
                      "be2")}

    in_maps = []
    for c in range(N_CORES):
        b, g = c // 4, c % 4
        idx = _row_index(g)
        xb = x[b]
        xrows = xb[idx]
        in_maps.append({
            "xT": np.ascontiguousarray(xb.T.astype(bf)),
            "xrT": np.ascontiguousarray(xrows.T.astype(bf)),
            "xr": np.ascontiguousarray(xrows),
            "wq": w_bf["Wq"], "wk": w_bf["Wk"], "wv": w_bf["Wv"],
            "wo": w_bf["Wo"], "w1": w_bf["W1"], "w2": w_bf["W2"],
            "bq": vecs["bq"], "bk": vecs["bk"],
            "bv": vecs["bv"].astype(bf), "bo": vecs["bo"].astype(bf),
            "b1": vecs["b1"], "b2": vecs["b2"].astype(bf),
            "g1": vecs["g1"].astype(bf), "be1": vecs["be1"].astype(bf),
            "g2": vecs["g2"].astype(bf), "be2": vecs["be2"].astype(bf),
            "cmask": _causal_masks(g).astype(bf),
        })

    res = run_bass_kernel_spmd(nc, in_maps, core_ids=list(range(N_CORES)))
    _CACHE["last_result"] = res

    outp = np.empty((B, L, D), dtype=np.float32)
    for c in range(N_CORES):
        b, g = c // 4, c % 4
        outp[b][_row_index(g)] = res.results[c]["out"].astype(np.float32)
    return outp


# revision 27
# speedup vs baseline: 1.0054x; 1.0054x over previous
"""Trainium2 Bass kernel for AttentionFFNBlock (B=2, L=2048, D=1024, H=16, FF=4096).

Sharding (8 cores, zero cross-core communication):
  core c -> batch b = c//4, group slot g = c%4.
  Each core owns 512 query rows of its batch, interleaved in 128-row blocks
  for causal load balance: global row = (2p+s)*512 + g*128 + i for local row
  r = p*256 + s*128 + i.  The core computes K/V for the full sequence
  (replicated inside the batch group), attention for its rows over all 16
  heads, then out-proj + LN1 + FFN + LN2 for its rows only.  Causality is
  enforced with per-core additive masks passed as input data (SPMD-safe).

Schedule: x arrives pre-transposed from the host (no DMA transposes); Q and
the first K/V chunks are projected up front; the remaining K / V projections
are interleaved into the attention head-pair loop so the PE stays dense
while the ACT engine works through the exp()s.  Scores skip dead (fully
masked) column blocks; head pairs share one exp instruction and alternate
PE row-groups (tile_position) so K=64 matmuls pack the array.  Wo/W1 are
prefetched as soon as SBUF frees up.  fc2 runs in two passes (rc pairs) so
the LN2 epilogues overlap the second pass's matmuls.

All matmuls in bf16 (fp32 PSUM accumulation); norms/softmax in fp32.
"""

import numpy as np
import ml_dtypes

import concourse.bass as bass
import concourse.mybir as mybir
import concourse.tile as tile
from concourse import bacc
from concourse.bass_utils import run_bass_kernel_spmd
from concourse.masks import make_identity

F32 = mybir.dt.float32
BF16 = mybir.dt.bfloat16
AF = mybir.ActivationFunctionType
ALU = mybir.AluOpType

N_CORES = 8
B, L, D = 2, 2048, 1024
H, HD = 16, 64
DFF = 4096
EPS = 1e-5
P = 128
NEG = -1e9

IC = D // P        # 8 contraction chunks of the model dim
TC = L // P        # 16 token chunks
FC = DFF // P      # 32 ff chunks
NPAIR = 8          # head pairs (= oc chunks)

_CACHE = {}


def _build():
    nc = bacc.Bacc("TRN2", target_bir_lowering=False, debug=False,
                   num_devices=N_CORES)

    def din(name, shape, dt=F32):
        return nc.dram_tensor(name, shape, dt, kind="ExternalInput").ap()

    io = dict(
        xT=din("xT", [D, L], BF16),               # x[b]^T (K/V source)
        xrT=din("xrT", [D, 512], BF16),           # owned rows^T (Q source)
        xr=din("xr", [512, D], F32),              # owned rows (residual)
        wq=din("wq", [D, D], BF16), wk=din("wk", [D, D], BF16),
        wv=din("wv", [D, D], BF16), wo=din("wo", [D, D], BF16),
        w1=din("w1", [D, DFF], BF16), w2=din("w2", [DFF, D], BF16),
        bq=din("bq", [D]), bk=din("bk", [D]), bv=din("bv", [D], BF16),
        bo=din("bo", [D], BF16), b1=din("b1", [DFF]), b2=din("b2", [D], BF16),
        g1=din("g1", [D], BF16), be1=din("be1", [D], BF16),
        g2=din("g2", [D], BF16), be2=din("be2", [D], BF16),
        cmask=din("cmask", [4, P, P], BF16),
        out=nc.dram_tensor("out", [512, D], BF16, kind="ExternalOutput").ap(),
    )

    with tile.TileContext(nc) as tc:
        _emit(nc, tc, io)
    nc.compile()
    return nc


def _layernorm(nc, pool, acc, eps_t, g_t, b_t, out_ap, g_eng=None,
               b_eng=None):
    """LayerNorm over the free axis (D=1024) of acc [128, 1024] -> out_ap."""
    stats = pool.tile([P, 2, 6], F32, tag="ln_stats")
    for sg in range(2):
        nc.vector.bn_stats(out=stats[:, sg, :], in_=acc[:, sg * 512:(sg + 1) * 512])
    mv = pool.tile([P, 2], F32, tag="ln_mv")
    nc.vector.bn_aggr(out=mv[:], in_=stats[:])
    rstd = pool.tile([P, 1], F32, tag="ln_rstd")
    nc.scalar.activation(out=rstd[:], in_=mv[:, 1:2], func=AF.Sqrt,
                         bias=eps_t[:], scale=1.0)
    nc.vector.reciprocal(out=rstd[:], in_=rstd[:])
    nmr = pool.tile([P, 1], F32, tag="ln_nmr")
    nc.vector.tensor_scalar(out=nmr[:], in0=mv[:, 0:1], scalar1=rstd[:],
                            scalar2=-1.0, op0=ALU.mult, op1=ALU.mult)
    u = pool.tile([P, D], BF16, tag="ln_u")
    nc.scalar.activation(out=u[:], in_=acc[:], func=AF.Identity,
                         bias=nmr[:], scale=rstd[:])
    (g_eng or nc.gpsimd).tensor_tensor(out=u[:], in0=u[:], in1=g_t[:, :],
                                       op=ALU.mult)
    (b_eng or nc.vector).tensor_tensor(out=out_ap, in0=u[:], in1=b_t[:, :],
                                       op=ALU.add)


def _emit(nc, tc, io):
    out = io["out"]

    with tc.tile_pool(name="const", bufs=1) as const:
        ao_pool = tc.alloc_tile_pool(name="ao_pool", bufs=1, side="right")
        # ---- constants / biases (tiles now; DMAs deferred past wk/xT) ----
        bq_t = const.tile([P, IC], F32)
        bk_t = const.tile([P, IC], F32)
        b1_t = const.tile([P, FC], F32)
        row_vecs = {}
        for nm in ("bv", "bo", "b2", "g1", "be1", "g2", "be2"):
            dt = F32 if nm.startswith("nope") else BF16
            rv = const.tile([P, D], dt, name=f"cv_{nm}")
            row_vecs[nm] = rv
        bv_t, bo_t, b2_t = row_vecs["bv"], row_vecs["bo"], row_vecs["b2"]
        g1_t, be1_t = row_vecs["g1"], row_vecs["be1"]
        g2_t, be2_t = row_vecs["g2"], row_vecs["be2"]
        cm_t = const.tile([P, 4, P], BF16)
        eps_t = const.tile([P, 1], F32)
        ident = const.tile([P, P], BF16)

        def early_dmas():
            nc.sync.dma_start(bq_t[:], io["bq"].rearrange("(o p) -> p o", p=P))
            nc.sync.dma_start(bk_t[:], io["bk"].rearrange("(o p) -> p o", p=P))
            nc.sync.dma_start(b1_t[:], io["b1"].rearrange("(f p) -> p f", p=P))
            nc.sync.dma_start(row_vecs["bv"][:],
                              io["bv"][None, :].to_broadcast([P, D]))
            nc.vector.memset(eps_t[:], EPS)

        def const_dmas():
            nc.sync.dma_start(cm_t[:], io["cmask"].rearrange("i p q -> p i q"))
            for nm in ("bo", "b2", "g1", "be1", "g2", "be2"):
                nc.sync.dma_start(row_vecs[nm][:],
                                  io[nm][None, :].to_broadcast([P, D]))
            make_identity(nc, ident[:])

        aoT = ao_pool.tile([P, IC, 512], BF16)   # attention output^T

        kv_pool = tc.alloc_tile_pool(name="kv_pool", bufs=1)
        ptile = tc.alloc_tile_pool(name="ptile", bufs=3)
        rtile = tc.alloc_tile_pool(name="rtile", bufs=2)
        spsum = tc.alloc_tile_pool(name="spsum", bufs=2, space="PSUM")
        avpsum = tc.alloc_tile_pool(name="avpsum", bufs=1, space="PSUM")
        if True:
            kT = kv_pool.tile([P, IC, L], BF16)
            v_all = kv_pool.tile([P, TC, H, HD + 1], BF16)
            qT = kv_pool.tile([P, IC, 512], BF16)
            nc.vector.memset(v_all[:, :, :, HD:], 1.0)

            proj_stream = []   # deferred (emit_mms, epilogue) generators

            def drain_proj(n):
                """Emit up to n deferred projection matmuls."""
                while n > 0 and proj_stream:
                    gen = proj_stream[0]
                    try:
                        next(gen)
                        n -= 1
                    except StopIteration:
                        proj_stream.pop(0)

            def attention(pair):
                oc = pair
                hA, hB = 2 * pair, 2 * pair + 1
                pavA = avpsum.tile([HD + 1, 512], F32, tag="avA")
                pavB = avpsum.tile([HD + 1, 512], F32, tag="avB")
                pts = []
                for kc in range(TC):
                    j0 = kc // 4
                    n0 = j0 * P
                    ps = spsum.tile([P, 2, 512], F32, tag="s")
                    nc.tensor.matmul(
                        ps[:, 0, n0:512],
                        kT[0:HD, oc, kc * P:(kc + 1) * P],
                        qT[0:HD, oc, n0:512], start=True, stop=True)
                    nc.tensor.matmul(
                        ps[:, 1, n0:512],
                        kT[HD:P, oc, kc * P:(kc + 1) * P],
                        qT[HD:P, oc, n0:512], start=True, stop=True)
                    pt = ptile.tile([P, 2, 512], BF16, tag="p")
                    nc.scalar.activation(out=pt[:, :, n0:512],
                                         in_=ps[:, :, n0:512],
                                         func=AF.Exp, scale=0.125)
                    # diagonal-window causal mask on block j0 (both heads):
                    # multiply by 0/1 post-exp (gpsimd cannot touch PSUM)
                    for j in range(2):
                        nc.gpsimd.tensor_tensor(
                            out=pt[:, j, n0:n0 + P], in0=pt[:, j, n0:n0 + P],
                            in1=cm_t[:, kc % 4, :], op=ALU.mult)
                    pts.append((kc, n0, pt))
                    drain_proj(4 if pair < 3 else 2)
                    # AV for the previous chunk (pipelined one deep)
                    if len(pts) >= 2:
                        pkc, pn0, ppt = pts.pop(0)
                        for j, (h, pav) in enumerate(((hA, pavA), (hB, pavB))):
                            nc.tensor.matmul(
                                pav[:, pn0:512], v_all[:, pkc, h, :],
                                ppt[:, j, pn0:512], start=(pkc == 0),
                                stop=False, skip_group_check=True)
                pkc, pn0, ppt = pts.pop(0)
                for j, (h, pav) in enumerate(((hA, pavA), (hB, pavB))):
                    nc.tensor.matmul(
                        pav[:, pn0:512], v_all[:, pkc, h, :],
                        ppt[:, j, pn0:512], start=False, stop=True,
                        skip_group_check=True)
                for hp, pav in ((0, pavA), (HD, pavB)):
                    rec = rtile.tile([1, 512], F32, tag="rec")
                    nc.vector.reciprocal(rec[:], pav[HD:HD + 1, :])
                    rec_b = rtile.tile([HD, 512], F32, tag="rec_b")
                    nc.gpsimd.partition_broadcast(rec_b[:], rec[0:1, :])
                    nc.vector.tensor_tensor(
                        out=aoT[hp:hp + HD, oc, :],
                        in0=pav[:HD, :], in1=rec_b[:], op=ALU.mult)

            # ---- projections (pairs 0..5 overlap with x_pool live) ----
            with (
                tc.tile_pool(name="x_pool", bufs=1) as x_pool,
                tc.tile_pool(name="ppsum", bufs=2, space="PSUM") as ppsum,
            ):
                wk_t = x_pool.tile([P, IC, D], BF16)
                xT_t = x_pool.tile([P, IC, L], BF16)
                wq_t = x_pool.tile([P, IC, D], BF16)
                xrT_t = x_pool.tile([P, IC, 512], BF16)
                wv_t = x_pool.tile([P, IC, D], BF16)
                wkr = io["wk"].rearrange("(i p) n -> p i n", p=P)
                wqr = io["wq"].rearrange("(i p) n -> p i n", p=P)
                wvr = io["wv"].rearrange("(i p) n -> p i n", p=P)
                xTr = io["xT"].rearrange("(i p) n -> p i n", p=P)
                early_dmas()
                nc.sync.dma_start(wk_t[:, :, 0:512], wkr[:, :, 0:512])
                nc.sync.dma_start(xT_t[:, :, 0:512], xTr[:, :, 0:512])
                nc.sync.dma_start(wq_t[:, :, 0:512], wqr[:, :, 0:512])
                nc.sync.dma_start(xrT_t[:],
                                  io["xrT"].rearrange("(i p) n -> p i n", p=P))
                nc.sync.dma_start(wv_t[:, :, 0:512], wvr[:, :, 0:512])
                nc.sync.dma_start(xT_t[:, :, 512:1024], xTr[:, :, 512:1024])
                nc.sync.dma_start(xT_t[:, :, 1024:1536], xTr[:, :, 1024:1536])
                nc.sync.dma_start(xT_t[:, :, 1536:2048], xTr[:, :, 1536:2048])
                nc.sync.dma_start(wk_t[:, :, 512:1024], wkr[:, :, 512:1024])
                nc.sync.dma_start(wq_t[:, :, 512:1024], wqr[:, :, 512:1024])
                nc.sync.dma_start(wv_t[:, :, 512:1024], wvr[:, :, 512:1024])
                const_dmas()

                def k_proj(oc):
                    for tcc in range(4):
                        ps = ppsum.tile([P, 512], F32, tag="proj")
                        for ic in range(IC):
                            nc.tensor.matmul(
                                ps[:], wk_t[:, ic, oc * P:(oc + 1) * P],
                                xT_t[:, ic, tcc * 512:(tcc + 1) * 512],
                                start=(ic == 0), stop=(ic == IC - 1))
                            yield
                        nc.vector.tensor_scalar_add(
                            out=kT[:, oc, tcc * 512:(tcc + 1) * 512],
                            in0=ps[:], scalar1=bk_t[:, oc:oc + 1])

                def q_proj(oc):
                    ps = ppsum.tile([P, 512], F32, tag="proj")
                    for ic in range(IC):
                        nc.tensor.matmul(
                            ps[:], wq_t[:, ic, oc * P:(oc + 1) * P],
                            xrT_t[:, ic, :],
                            start=(ic == 0), stop=(ic == IC - 1))
                        yield
                    nc.vector.tensor_scalar_add(
                        out=qT[:, oc, :], in0=ps[:], scalar1=bq_t[:, oc:oc + 1])

                def v_proj(tcc, hf):
                    ps = ppsum.tile([P, 512], F32, tag="proj")
                    for ic in range(IC):
                        nc.tensor.matmul(
                            ps[:], xT_t[:, ic, tcc * P:(tcc + 1) * P],
                            wv_t[:, ic, hf * 512:(hf + 1) * 512],
                            start=(ic == 0), stop=(ic == IC - 1))
                        yield
                    nc.vector.tensor_tensor(
                        out=v_all[:, tcc, hf * 8:(hf + 1) * 8, :HD],
                        in0=ps.rearrange("p (h d) -> p h d", d=HD),
                        in1=bv_t[:, hf * 512:(hf + 1) * 512]
                        .rearrange("p (h d) -> p h d", d=HD),
                        op=ALU.add)

                # upfront, ordered to match serial DMA arrival
                def adv(gen, n):
                    for _ in range(n):
                        try:
                            next(gen)
                        except StopIteration:
                            return
                k0, k1 = k_proj(0), k_proj(1)
                qs = [q_proj(oc) for oc in range(IC)]
                v0s = [v_proj(tcc, 0) for tcc in range(TC)]
                adv(k0, 8)                       # K0.tcc0 (wk0+xT0)
                for oc in range(4):
                    adv(qs[oc], 9)               # Q0-3 (wq0+xrT)
                for tcc in range(4):
                    adv(v0s[tcc], 9)             # V0 tcc0-3 (wv0+xT0)
                adv(k0, 100)                     # K0 rest (xT1-3)
                adv(k1, 32)                      # K1 (wk1)
                for oc in range(4, IC):
                    adv(qs[oc], 9)               # Q4-7 (wq1)
                for tcc in range(4, TC):
                    adv(v0s[tcc], 9)             # V0 rest
                for g in [k0, k1] + qs + v0s:
                    adv(g, 100)
                # deferred: K2,K3, all of V1, K4..K7 — drained inside attention
                proj_stream.extend([k_proj(2), k_proj(3)])
                proj_stream.extend(v_proj(tcc, 1) for tcc in range(TC))
                proj_stream.extend(k_proj(oc) for oc in range(4, IC))

                for pair in range(7):
                    attention(pair)
                drain_proj(1 << 30)

            # x_pool freed: prefetch xr + wo under attn 7 (right side)
            xrr_pool = tc.alloc_tile_pool(name="xrr_pool", bufs=1, side="right")
            xr_nat = xrr_pool.tile([P, 4, D], F32)
            nc.sync.dma_start(xr_nat[:],
                              io["xr"].rearrange("(rc p) d -> p rc d", p=P))
            wo_pool = tc.alloc_tile_pool(name="wo_pool", bufs=1, side="right")
            wo_t = wo_pool.tile([P, IC, D], BF16)
            wor = io["wo"].rearrange("(i p) n -> p i n", p=P)
            for h2 in range(2):
                nc.sync.dma_start(wo_t[:, :, h2 * 512:(h2 + 1) * 512],
                                  wor[:, :, h2 * 512:(h2 + 1) * 512])

            attention(7)

            # free the attention pools (non-LIFO: wo/w1a stay live)
            avpsum.release()
            spsum.release()
            rtile.release()
            ptile.release()
            kv_pool.release()

            w1_pool = tc.alloc_tile_pool(name="w1_pool", bufs=1)
            w1_t = w1_pool.tile([P, IC, DFF], BF16)
            w1r = io["w1"].rearrange("(i p) n -> p i n", p=P)
            for c in range(8):
                nc.sync.dma_start(w1_t[:, :, c * 512:(c + 1) * 512],
                                  w1r[:, :, c * 512:(c + 1) * 512])


            if True:
                # ---- out-proj + LN1 + transpose ----
                with tc.tile_pool(name="t_pool", bufs=1) as t_pool:
                    tbf = t_pool.tile([P, 4, D], BF16)    # LN1 out (residual)
                    tT = t_pool.tile([P, IC, 512], BF16)  # LN1 out transposed

                    with (
                        tc.tile_pool(name="lnt", bufs=4) as lnt,
                        tc.tile_pool(name="opsum", bufs=4, space="PSUM") as opsum,
                        tc.tile_pool(name="trpsum", bufs=4, space="PSUM") as trpsum,
                    ):
                        for rc in range(4):
                            acc = lnt.tile([P, D], F32, tag="acc")
                            for n2 in range(2):
                                ps = opsum.tile([P, 512], F32, tag="o")
                                for dc in range(IC):
                                    nc.tensor.matmul(
                                        ps[:], aoT[:, dc, rc * P:(rc + 1) * P],
                                        wo_t[:, dc, n2 * 512:(n2 + 1) * 512],
                                        start=(dc == 0), stop=(dc == IC - 1))
                                nc.vector.tensor_tensor(
                                    out=acc[:, n2 * 512:(n2 + 1) * 512],
                                    in0=ps[:],
                                    in1=xr_nat[:, rc, n2 * 512:(n2 + 1) * 512],
                                    op=ALU.add)
                            nc.vector.tensor_tensor(
                                out=acc[:], in0=acc[:], in1=bo_t[:, :],
                                op=ALU.add)
                            _layernorm(nc, lnt, acc, eps_t, g1_t, be1_t,
                                       tbf[:, rc, :])
                        for rc in range(4):
                            for ic in range(IC):
                                pst = trpsum.tile([P, P], BF16, tag="tr")
                                nc.tensor.transpose(
                                    pst[:], tbf[:, rc, ic * P:(ic + 1) * P],
                                    ident[:])
                                if ic % 2 == 0:
                                    nc.vector.tensor_copy(
                                        tT[:, ic, rc * P:(rc + 1) * P], pst[:])
                                else:
                                    nc.scalar.copy(
                                        tT[:, ic, rc * P:(rc + 1) * P], pst[:])

                    wo_pool.release()
                    xrr_pool.release()
                    ao_pool.release()

                    # ================= FFN =================
                    w2_pool = tc.alloc_tile_pool(name="w2_pool", bufs=1)
                    w2_t = w2_pool.tile([P, FC, D], BF16)
                    w2r = io["w2"].rearrange("(f p) n -> p f n", p=P)
                    for grp in range(8):
                        nc.sync.dma_start(w2_t[:, grp * 4:(grp + 1) * 4, :],
                                          w2r[:, grp * 4:(grp + 1) * 4, :])
                    with (
                        tc.tile_pool(name="h_pool", bufs=1) as h_pool,
                        tc.tile_pool(name="fpsum", bufs=2, space="PSUM") as fpsum,
                        tc.tile_pool(name="ypsum", bufs=3, space="PSUM") as ypsum,
                    ):
                        hT = h_pool.tile([P, FC, 512], BF16)
                        psy = {}

                        def fc2_mms(fc, rcs):
                            for rc in rcs:
                                for n2 in range(2):
                                    nc.tensor.matmul(
                                        psy[rc][:, n2, :],
                                        hT[:, fc, rc * P:(rc + 1) * P],
                                        w2_t[:, fc, n2 * 512:(n2 + 1) * 512],
                                        start=(fc == 0), stop=(fc == FC - 1))

                        finbox = {}

                        def epilogue(rc):
                            fin = finbox["p"]
                            acc = fin.tile([P, D], F32, tag="acc2", bufs=2)
                            for n2 in range(2):
                                nc.vector.tensor_tensor(
                                    out=acc[:, n2 * 512:(n2 + 1) * 512],
                                    in0=psy[rc][:, n2, :],
                                    in1=tbf[:, rc, n2 * 512:(n2 + 1) * 512],
                                    op=ALU.add)
                            nc.vector.tensor_tensor(
                                out=acc[:], in0=acc[:], in1=b2_t[:, :],
                                op=ALU.add)
                            res = fin.tile([P, D], BF16, tag="res", bufs=2)
                            _layernorm(nc, fin, acc, eps_t, g2_t, be2_t,
                                       res[:], g_eng=nc.vector,
                                       b_eng=nc.vector)
                            nc.sync.dma_start(
                                out.rearrange("(rc p) d -> p rc d", p=P)[:, rc, :],
                                res[:])

                        # pass 1: fc1 + fc2 for rc 0,1,2 interleaved per fc
                        psy[0] = ypsum.tile([P, 2, 512], F32, tag="y", name="psy0")
                        psy[1] = ypsum.tile([P, 2, 512], F32, tag="y", name="psy1")
                        psy[2] = ypsum.tile([P, 2, 512], F32, tag="y", name="psy2")
                        for grp in range(8):
                            for k in range(4):
                                fc = grp * 4 + k
                                ps = fpsum.tile([P, 512], F32, tag="f1")
                                for ic in range(IC):
                                    nc.tensor.matmul(
                                        ps[:],
                                        w1_t[:, ic, fc * P:(fc + 1) * P],
                                        tT[:, ic, :],
                                        start=(ic == 0), stop=(ic == IC - 1))
                                nc.scalar.activation(out=hT[:, fc, :], in_=ps[:],
                                                     func=AF.Gelu,
                                                     bias=b1_t[:, fc:fc + 1],
                                                     scale=1.0)
                                fc2_mms(fc, (0, 1, 2))
                        finbox["p"] = tc.alloc_tile_pool(name="fin", bufs=1)
                        epilogue(0)
                        epilogue(1)
                        epilogue(2)
                        # pass 2: fc2 for rc3 (w2 already prefetched)
                        psy[3] = ypsum.tile([P, 2, 512], F32, tag="y", name="psy3")
                        for fc in range(FC):
                            fc2_mms(fc, (3,))
                        epilogue(3)
                        finbox["p"].release()
                    w2_pool.release()

            w1_pool.release()


def _row_index(g):
    idx = np.empty(512, dtype=np.int64)
    r = 0
    for p in range(2):
        for s in range(2):
            j = 2 * p + s
            base = j * 512 + g * 128
            idx[r:r + 128] = np.arange(base, base + 128)
            r += 128
    return idx


def _causal_masks(g):
    kj = np.arange(P)[:, None]
    qi = np.arange(P)[None, :]
    m = np.empty((4, P, P), dtype=np.float32)
    for i in range(4):
        m[i] = np.where(kj <= qi + (g - i) * P, 1.0, 0.0)
    return m


def kernel(**inputs):
    if "nc" not in _CACHE:
        _CACHE["nc"] = _build()
    nc = _CACHE["nc"]

    bf = ml_dtypes.bfloat16
    x = np.asarray(inputs["x"], dtype=np.float32)
    w_bf = {k: np.ascontiguousarray(
        np.asarray(inputs[k], dtype=np.float32).astype(bf))
        for k in ("Wq", "Wk", "Wv", "Wo", "W1", "W2")}
    vecs = {k: np.ascontiguousarray(np.asarray(inputs[k], dtype=np.float32))
            for k in ("bq", "bk", "bv", "bo", "b1", "b2", "g1", "be1", "g2",
                      "be2")}

    in_maps = []
    for c in range(N_CORES):
        b, g = c // 4, c % 4
        idx = _row_index(g)
        xb = x[b]
        xrows = xb[idx]
        in_maps.append({
            "xT": np.ascontiguousarray(xb.T.astype(bf)),
            "xrT": np.ascontiguousarray(xrows.T.astype(bf)),
            "xr": np.ascontiguousarray(xrows),
            "wq": w_bf["Wq"], "wk": w_bf["Wk"], "wv": w_bf["Wv"],
            "wo": w_bf["Wo"], "w1": w_bf["W1"], "w2": w_bf["W2"],
            "bq": vecs["bq"], "bk": vecs["bk"],
            "bv": vecs["bv"].astype(bf), "bo": vecs["bo"].astype(bf),
            "b1": vecs["b1"], "b2": vecs["b2"].astype(bf),
            "g1": vecs["g1"].astype(bf), "be1": vecs["be1"].astype(bf),
            "g2": vecs["g2"].astype(bf), "be2": vecs["be2"].astype(bf),
            "cmask": _causal_masks(g).astype(bf),
        })

    res = run_bass_kernel_spmd(nc, in_maps, core_ids=list(range(N_CORES)))
    _CACHE["last_result"] = res

    outp = np.empty((B, L, D), dtype=np.float32)
    for c in range(N_CORES):
        b, g = c // 4, c % 4
        outp[b][_row_index(g)] = res.results[c]["out"].astype(np.float32)
    return outp


# revision 32
# speedup vs baseline: 1.0138x; 1.0083x over previous
"""Trainium2 Bass kernel for AttentionFFNBlock (B=2, L=2048, D=1024, H=16, FF=4096).

Sharding (8 cores, zero cross-core communication):
  core c -> batch b = c//4, group slot g = c%4.
  Each core owns 512 query rows of its batch, interleaved in 128-row blocks
  for causal load balance: global row = (2p+s)*512 + g*128 + i for local row
  r = p*256 + s*128 + i.  The core computes K/V for the full sequence
  (replicated inside the batch group), attention for its rows over all 16
  heads, then out-proj + LN1 + FFN + LN2 for its rows only.  Causality is
  enforced with per-core additive masks passed as input data (SPMD-safe).

Schedule: x arrives pre-transposed from the host (no DMA transposes); Q and
the first K/V chunks are projected up front; the remaining K / V projections
are interleaved into the attention head-pair loop so the PE stays dense
while the ACT engine works through the exp()s.  Scores skip dead (fully
masked) column blocks; head pairs share one exp instruction and alternate
PE row-groups (tile_position) so K=64 matmuls pack the array.  Wo/W1 are
prefetched as soon as SBUF frees up.  fc2 runs in two passes (rc pairs) so
the LN2 epilogues overlap the second pass's matmuls.

All matmuls in bf16 (fp32 PSUM accumulation); norms/softmax in fp32.
"""

import numpy as np
import ml_dtypes

import concourse.bass as bass
import concourse.mybir as mybir
import concourse.tile as tile
from concourse import bacc
from concourse.bass_utils import run_bass_kernel_spmd
from concourse.masks import make_identity

F32 = mybir.dt.float32
BF16 = mybir.dt.bfloat16
AF = mybir.ActivationFunctionType
ALU = mybir.AluOpType

N_CORES = 8
B, L, D = 2, 2048, 1024
H, HD = 16, 64
DFF = 4096
EPS = 1e-5
P = 128
NEG = -1e9

IC = D // P        # 8 contraction chunks of the model dim
TC = L // P        # 16 token chunks
FC = DFF // P      # 32 ff chunks
NPAIR = 8          # head pairs (= oc chunks)

_CACHE = {}


def _build():
    nc = bacc.Bacc("TRN2", target_bir_lowering=False, debug=False,
                   num_devices=N_CORES)

    def din(name, shape, dt=F32):
        return nc.dram_tensor(name, shape, dt, kind="ExternalInput").ap()

    io = dict(
        xT=din("xT", [D, L], BF16),               # x[b]^T (K/V source)
        xrT=din("xrT", [D, 512], BF16),           # owned rows^T (Q source)
        xr=din("xr", [512, D], F32),              # owned rows (residual)
        wq=din("wq", [D, D], BF16), wk=din("wk", [D, D], BF16),
        wv=din("wv", [D, D], BF16), wo=din("wo", [D, D], BF16),
        w1=din("w1", [D, DFF], BF16), w2=din("w2", [DFF, D], BF16),
        bq=din("bq", [D]), bk=din("bk", [D]), bv=din("bv", [D], BF16),
        bo=din("bo", [D], BF16), b1=din("b1", [DFF]), b2=din("b2", [D], BF16),
        g1=din("g1", [D], BF16), be1=din("be1", [D], BF16),
        g2=din("g2", [D], BF16), be2=din("be2", [D], BF16),
        cmask=din("cmask", [4, P, P], BF16),
        out=nc.dram_tensor("out", [512, D], BF16, kind="ExternalOutput").ap(),
    )

    with tile.TileContext(nc) as tc:
        _emit(nc, tc, io)
    nc.compile()
    return nc


def _layernorm(nc, pool, acc, eps_t, g_t, b_t, out_ap, g_eng=None,
               b_eng=None):
    """LayerNorm over the free axis (D=1024) of acc [128, 1024] -> out_ap."""
    stats = pool.tile([P, 2, 6], F32, tag="ln_stats")
    for sg in range(2):
        nc.vector.bn_stats(out=stats[:, sg, :], in_=acc[:, sg * 512:(sg + 1) * 512])
    mv = pool.tile([P, 2], F32, tag="ln_mv")
    nc.vector.bn_aggr(out=mv[:], in_=stats[:])
    rstd = pool.tile([P, 1], F32, tag="ln_rstd")
    nc.scalar.activation(out=rstd[:], in_=mv[:, 1:2], func=AF.Sqrt,
                         bias=eps_t[:], scale=1.0)
    nc.vector.reciprocal(out=rstd[:], in_=rstd[:])
    nmr = pool.tile([P, 1], F32, tag="ln_nmr")
    nc.vector.tensor_scalar(out=nmr[:], in0=mv[:, 0:1], scalar1=rstd[:],
                            scalar2=-1.0, op0=ALU.mult, op1=ALU.mult)
    u = pool.tile([P, D], BF16, tag="ln_u")
    nc.scalar.activation(out=u[:], in_=acc[:], func=AF.Identity,
                         bias=nmr[:], scale=rstd[:])
    (g_eng or nc.gpsimd).tensor_tensor(out=u[:], in0=u[:], in1=g_t[:, :],
                                       op=ALU.mult)
    (b_eng or nc.vector).tensor_tensor(out=out_ap, in0=u[:], in1=b_t[:, :],
                                       op=ALU.add)


def _emit(nc, tc, io):
    out = io["out"]

    with tc.tile_pool(name="const", bufs=1) as const:
        ao_pool = tc.alloc_tile_pool(name="ao_pool", bufs=1, side="right")
        # ---- constants / biases (tiles now; DMAs deferred past wk/xT) ----
        bq_t = const.tile([P, IC], F32)
        bk_t = const.tile([P, IC], F32)
        b1_t = const.tile([P, FC], F32)
        row_vecs = {}
        for nm in ("bv", "bo", "b2", "g1", "be1", "g2", "be2"):
            dt = F32 if nm.startswith("nope") else BF16
            rv = const.tile([P, D], dt, name=f"cv_{nm}")
            row_vecs[nm] = rv
        bv_t, bo_t, b2_t = row_vecs["bv"], row_vecs["bo"], row_vecs["b2"]
        g1_t, be1_t = row_vecs["g1"], row_vecs["be1"]
        g2_t, be2_t = row_vecs["g2"], row_vecs["be2"]
        cm_t = const.tile([P, 4, P], BF16)
        eps_t = const.tile([P, 1], F32)
        ident = const.tile([P, P], BF16)

        def early_dmas():
            nc.sync.dma_start(bq_t[:], io["bq"].rearrange("(o p) -> p o", p=P))
            nc.sync.dma_start(bk_t[:], io["bk"].rearrange("(o p) -> p o", p=P))
            nc.sync.dma_start(b1_t[:], io["b1"].rearrange("(f p) -> p f", p=P))
            nc.sync.dma_start(row_vecs["bv"][:],
                              io["bv"][None, :].to_broadcast([P, D]))
            nc.vector.memset(eps_t[:], EPS)

        def const_dmas():
            nc.sync.dma_start(cm_t[:], io["cmask"].rearrange("i p q -> p i q"))
            for nm in ("bo", "b2", "g1", "be1", "g2", "be2"):
                nc.sync.dma_start(row_vecs[nm][:],
                                  io[nm][None, :].to_broadcast([P, D]))
            make_identity(nc, ident[:])

        aoT = ao_pool.tile([P, IC, 512], BF16)   # attention output^T

        kv_pool = tc.alloc_tile_pool(name="kv_pool", bufs=1)
        ptile = tc.alloc_tile_pool(name="ptile", bufs=3)
        rtile = tc.alloc_tile_pool(name="rtile", bufs=2)
        spsum = tc.alloc_tile_pool(name="spsum", bufs=2, space="PSUM")
        avpsum = tc.alloc_tile_pool(name="avpsum", bufs=1, space="PSUM")
        if True:
            kT = kv_pool.tile([P, IC, L], BF16)
            v_all = kv_pool.tile([P, TC, H, HD + 1], BF16)
            qT = kv_pool.tile([P, IC, 512], BF16)
            nc.vector.memset(v_all[:, :, :, HD:], 1.0)

            proj_stream = []   # deferred (emit_mms, epilogue) generators

            def drain_proj(n):
                """Emit up to n deferred projection matmuls."""
                while n > 0 and proj_stream:
                    gen = proj_stream[0]
                    try:
                        next(gen)
                        n -= 1
                    except StopIteration:
                        proj_stream.pop(0)

            def attention(pair):
                oc = pair
                hA, hB = 2 * pair, 2 * pair + 1
                pavA = avpsum.tile([HD + 1, 512], F32, tag="avA")
                pavB = avpsum.tile([HD + 1, 512], F32, tag="avB")
                pts = []
                for kc in range(TC):
                    j0 = kc // 4
                    n0 = j0 * P
                    ps = spsum.tile([P, 2, 512], F32, tag="s")
                    nc.tensor.matmul(
                        ps[:, 0, n0:512],
                        kT[0:HD, oc, kc * P:(kc + 1) * P],
                        qT[0:HD, oc, n0:512], start=True, stop=True)
                    nc.tensor.matmul(
                        ps[:, 1, n0:512],
                        kT[HD:P, oc, kc * P:(kc + 1) * P],
                        qT[HD:P, oc, n0:512], start=True, stop=True)
                    pt = ptile.tile([P, 2, 512], BF16, tag="p")
                    nc.scalar.activation(out=pt[:, :, n0:512],
                                         in_=ps[:, :, n0:512],
                                         func=AF.Exp, scale=0.125)
                    # diagonal-window causal mask on block j0 (both heads):
                    # multiply by 0/1 post-exp (gpsimd cannot touch PSUM)
                    for j in range(2):
                        nc.gpsimd.tensor_tensor(
                            out=pt[:, j, n0:n0 + P], in0=pt[:, j, n0:n0 + P],
                            in1=cm_t[:, kc % 4, :], op=ALU.mult)
                    pts.append((kc, n0, pt))
                    drain_proj(4 if pair < 3 else 2)
                    # AV for the previous chunk (pipelined one deep)
                    if len(pts) >= 2:
                        pkc, pn0, ppt = pts.pop(0)
                        for j, (h, pav) in enumerate(((hA, pavA), (hB, pavB))):
                            nc.tensor.matmul(
                                pav[:, pn0:512], v_all[:, pkc, h, :],
                                ppt[:, j, pn0:512], start=(pkc == 0),
                                stop=False, skip_group_check=True)
                pkc, pn0, ppt = pts.pop(0)
                for j, (h, pav) in enumerate(((hA, pavA), (hB, pavB))):
                    nc.tensor.matmul(
                        pav[:, pn0:512], v_all[:, pkc, h, :],
                        ppt[:, j, pn0:512], start=False, stop=True,
                        skip_group_check=True)
                for hp, pav in ((0, pavA), (HD, pavB)):
                    rec = rtile.tile([1, 512], F32, tag="rec")
                    nc.vector.reciprocal(rec[:], pav[HD:HD + 1, :])
                    rec_b = rtile.tile([HD, 512], F32, tag="rec_b")
                    nc.gpsimd.partition_broadcast(rec_b[:], rec[0:1, :])
                    nc.vector.tensor_tensor(
                        out=aoT[hp:hp + HD, oc, :],
                        in0=pav[:HD, :], in1=rec_b[:], op=ALU.mult)

            # ---- projections (pairs 0..5 overlap with x_pool live) ----
            with (
                tc.tile_pool(name="x_pool", bufs=1) as x_pool,
                tc.tile_pool(name="ppsum", bufs=2, space="PSUM") as ppsum,
            ):
                wk_t = x_pool.tile([P, IC, D], BF16)
                xT_t = x_pool.tile([P, IC, L], BF16)
                wq_t = x_pool.tile([P, IC, D], BF16)
                xrT_t = x_pool.tile([P, IC, 512], BF16)
                wv_t = x_pool.tile([P, IC, D], BF16)
                wkr = io["wk"].rearrange("(i p) n -> p i n", p=P)
                wqr = io["wq"].rearrange("(i p) n -> p i n", p=P)
                wvr = io["wv"].rearrange("(i p) n -> p i n", p=P)
                xTr = io["xT"].rearrange("(i p) n -> p i n", p=P)
                nc.sync.dma_start(wk_t[:, :, 0:P], wkr[:, :, 0:P])
                nc.sync.dma_start(xT_t[:, 0:4, 0:512], xTr[:, 0:4, 0:512])
                nc.sync.dma_start(xT_t[:, 4:8, 0:512], xTr[:, 4:8, 0:512])
                nc.sync.dma_start(wq_t[:, :, 0:512], wqr[:, :, 0:512])
                nc.sync.dma_start(xrT_t[:],
                                  io["xrT"].rearrange("(i p) n -> p i n", p=P))
                nc.sync.dma_start(wv_t[:, :, 0:512], wvr[:, :, 0:512])
                early_dmas()
                nc.sync.dma_start(xT_t[:, :, 512:1024], xTr[:, :, 512:1024])
                nc.sync.dma_start(wk_t[:, :, P:512], wkr[:, :, P:512])
                nc.sync.dma_start(xT_t[:, :, 1024:1536], xTr[:, :, 1024:1536])
                nc.sync.dma_start(xT_t[:, :, 1536:2048], xTr[:, :, 1536:2048])
                nc.sync.dma_start(wk_t[:, :, 512:1024], wkr[:, :, 512:1024])
                nc.sync.dma_start(wq_t[:, :, 512:1024], wqr[:, :, 512:1024])
                const_dmas()
                nc.sync.dma_start(wv_t[:, :, 512:1024], wvr[:, :, 512:1024])

                def k_proj(oc):
                    for tcc in range(4):
                        ps = ppsum.tile([P, 512], F32, tag="proj")
                        for ic in range(IC):
                            nc.tensor.matmul(
                                ps[:], wk_t[:, ic, oc * P:(oc + 1) * P],
                                xT_t[:, ic, tcc * 512:(tcc + 1) * 512],
                                start=(ic == 0), stop=(ic == IC - 1))
                            yield
                        nc.vector.tensor_scalar_add(
                            out=kT[:, oc, tcc * 512:(tcc + 1) * 512],
                            in0=ps[:], scalar1=bk_t[:, oc:oc + 1])

                def q_proj(oc):
                    ps = ppsum.tile([P, 512], F32, tag="proj")
                    for ic in range(IC):
                        nc.tensor.matmul(
                            ps[:], wq_t[:, ic, oc * P:(oc + 1) * P],
                            xrT_t[:, ic, :],
                            start=(ic == 0), stop=(ic == IC - 1))
                        yield
                    nc.vector.tensor_scalar_add(
                        out=qT[:, oc, :], in0=ps[:], scalar1=bq_t[:, oc:oc + 1])

                def v_proj(tcc, hf):
                    ps = ppsum.tile([P, 512], F32, tag="proj")
                    for ic in range(IC):
                        nc.tensor.matmul(
                            ps[:], xT_t[:, ic, tcc * P:(tcc + 1) * P],
                            wv_t[:, ic, hf * 512:(hf + 1) * 512],
                            start=(ic == 0), stop=(ic == IC - 1))
                        yield
                    nc.vector.tensor_tensor(
                        out=v_all[:, tcc, hf * 8:(hf + 1) * 8, :HD],
                        in0=ps.rearrange("p (h d) -> p h d", d=HD),
                        in1=bv_t[:, hf * 512:(hf + 1) * 512]
                        .rearrange("p (h d) -> p h d", d=HD),
                        op=ALU.add)

                # upfront, ordered to match serial DMA arrival
                def adv(gen, n):
                    for _ in range(n):
                        try:
                            next(gen)
                        except StopIteration:
                            return
                k0, k1 = k_proj(0), k_proj(1)
                qs = [q_proj(oc) for oc in range(IC)]
                v0s = [v_proj(tcc, 0) for tcc in range(TC)]
                adv(k0, 8)                       # K0.tcc0 (wk0+xT0)
                for oc in range(4):
                    adv(qs[oc], 9)               # Q0-3 (wq0+xrT)
                for tcc in range(4):
                    adv(v0s[tcc], 9)             # V0 tcc0-3 (wv0+xT0)
                adv(k0, 100)                     # K0 rest (xT1-3)
                adv(k1, 32)                      # K1 (wk1)
                for oc in range(4, IC):
                    adv(qs[oc], 9)               # Q4-7 (wq1)
                for tcc in range(4, TC):
                    adv(v0s[tcc], 9)             # V0 rest
                for g in [k0, k1] + qs + v0s:
                    adv(g, 100)
                # deferred: K2,K3, all of V1, K4..K7 — drained inside attention
                proj_stream.extend([k_proj(2), k_proj(3)])
                proj_stream.extend(v_proj(tcc, 1) for tcc in range(TC))
                proj_stream.extend(k_proj(oc) for oc in range(4, IC))

                for pair in range(7):
                    attention(pair)
                drain_proj(1 << 30)

            # x_pool freed: prefetch xr + wo under attn 7 (right side)
            xrr_pool = tc.alloc_tile_pool(name="xrr_pool", bufs=1, side="right")
            xr_nat = xrr_pool.tile([P, 4, D], F32)
            nc.sync.dma_start(xr_nat[:],
                              io["xr"].rearrange("(rc p) d -> p rc d", p=P))
            wo_pool = tc.alloc_tile_pool(name="wo_pool", bufs=1, side="right")
            wo_t = wo_pool.tile([P, IC, D], BF16)
            wor = io["wo"].rearrange("(i p) n -> p i n", p=P)
            for h2 in range(2):
                nc.sync.dma_start(wo_t[:, :, h2 * 512:(h2 + 1) * 512],
                                  wor[:, :, h2 * 512:(h2 + 1) * 512])

            attention(7)

            # free the attention pools (non-LIFO: wo/w1a stay live)
            avpsum.release()
            spsum.release()
            rtile.release()
            ptile.release()
            kv_pool.release()

            w1_pool = tc.alloc_tile_pool(name="w1_pool", bufs=1)
            w1_t = w1_pool.tile([P, IC, DFF], BF16)
            w1r = io["w1"].rearrange("(i p) n -> p i n", p=P)
            for c in range(8):
                nc.sync.dma_start(w1_t[:, :, c * 512:(c + 1) * 512],
                                  w1r[:, :, c * 512:(c + 1) * 512])


            if True:
                # ---- out-proj + LN1 + transpose ----
                with tc.tile_pool(name="t_pool", bufs=1) as t_pool:
                    tbf = t_pool.tile([P, 4, D], BF16)    # LN1 out (residual)
                    tT = t_pool.tile([P, IC, 512], BF16)  # LN1 out transposed

                    with (
                        tc.tile_pool(name="lnt", bufs=4) as lnt,
                        tc.tile_pool(name="opsum", bufs=4, space="PSUM") as opsum,
                        tc.tile_pool(name="trpsum", bufs=4, space="PSUM") as trpsum,
                    ):
                        for rc in range(4):
                            acc = lnt.tile([P, D], F32, tag="acc")
                            for n2 in range(2):
                                ps = opsum.tile([P, 512], F32, tag="o")
                                for dc in range(IC):
                                    nc.tensor.matmul(
                                        ps[:], aoT[:, dc, rc * P:(rc + 1) * P],
                                        wo_t[:, dc, n2 * 512:(n2 + 1) * 512],
                                        start=(dc == 0), stop=(dc == IC - 1))
                                nc.vector.tensor_tensor(
                                    out=acc[:, n2 * 512:(n2 + 1) * 512],
                                    in0=ps[:],
                                    in1=xr_nat[:, rc, n2 * 512:(n2 + 1) * 512],
                                    op=ALU.add)
                            nc.vector.tensor_tensor(
                                out=acc[:], in0=acc[:], in1=bo_t[:, :],
                                op=ALU.add)
                            _layernorm(nc, lnt, acc, eps_t, g1_t, be1_t,
                                       tbf[:, rc, :])
                        for rc in range(4):
                            for ic in range(IC):
                                pst = trpsum.tile([P, P], BF16, tag="tr")
                                nc.tensor.transpose(
                                    pst[:], tbf[:, rc, ic * P:(ic + 1) * P],
                                    ident[:])
                                if ic % 2 == 0:
                                    nc.vector.tensor_copy(
                                        tT[:, ic, rc * P:(rc + 1) * P], pst[:])
                                else:
                                    nc.scalar.copy(
                                        tT[:, ic, rc * P:(rc + 1) * P], pst[:])

                    wo_pool.release()
                    xrr_pool.release()
                    ao_pool.release()

                    # ================= FFN =================
                    w2_pool = tc.alloc_tile_pool(name="w2_pool", bufs=1)
                    w2_t = w2_pool.tile([P, FC, D], BF16)
                    w2r = io["w2"].rearrange("(f p) n -> p f n", p=P)
                    for grp in range(8):
                        nc.sync.dma_start(w2_t[:, grp * 4:(grp + 1) * 4, :],
                                          w2r[:, grp * 4:(grp + 1) * 4, :])
                    with (
                        tc.tile_pool(name="h_pool", bufs=1) as h_pool,
                        tc.tile_pool(name="fpsum", bufs=2, space="PSUM") as fpsum,
                        tc.tile_pool(name="ypsum", bufs=3, space="PSUM") as ypsum,
                    ):
                        hT = h_pool.tile([P, FC, 512], BF16)
                        psy = {}

                        def fc2_mms(fc, rcs):
                            for rc in rcs:
                                for n2 in range(2):
                                    nc.tensor.matmul(
                                        psy[rc][:, n2, :],
                                        hT[:, fc, rc * P:(rc + 1) * P],
                                        w2_t[:, fc, n2 * 512:(n2 + 1) * 512],
                                        start=(fc == 0), stop=(fc == FC - 1))

                        finbox = {}

                        def epilogue(rc):
                            fin = finbox["p"]
                            acc = fin.tile([P, D], F32, tag="acc2", bufs=2)
                            for n2 in range(2):
                                nc.vector.tensor_tensor(
                                    out=acc[:, n2 * 512:(n2 + 1) * 512],
                                    in0=psy[rc][:, n2, :],
                                    in1=tbf[:, rc, n2 * 512:(n2 + 1) * 512],
                                    op=ALU.add)
                            nc.vector.tensor_tensor(
                                out=acc[:], in0=acc[:], in1=b2_t[:, :],
                                op=ALU.add)
                            res = fin.tile([P, D], BF16, tag="res", bufs=2)
                            _layernorm(nc, fin, acc, eps_t, g2_t, be2_t,
                                       res[:], g_eng=nc.vector,
                                       b_eng=nc.vector)
                            nc.sync.dma_start(
                                out.rearrange("(rc p) d -> p rc d", p=P)[:, rc, :],
                                res[:])

                        # pass 1: fc1 + fc2 for rc 0,1,2 interleaved per fc
                        psy[0] = ypsum.tile([P, 2, 512], F32, tag="y", name="psy0")
                        psy[1] = ypsum.tile([P, 2, 512], F32, tag="y", name="psy1")
                        psy[2] = ypsum.tile([P, 2, 512], F32, tag="y", name="psy2")
                        for grp in range(8):
                            for k in range(4):
                                fc = grp * 4 + k
                                ps = fpsum.tile([P, 512], F32, tag="f1")
                                for ic in range(IC):
                                    nc.tensor.matmul(
                                        ps[:],
                                        w1_t[:, ic, fc * P:(fc + 1) * P],
                                        tT[:, ic, :],
                                        start=(ic == 0), stop=(ic == IC - 1))
                                nc.scalar.activation(out=hT[:, fc, :], in_=ps[:],
                                                     func=AF.Gelu,
                                                     bias=b1_t[:, fc:fc + 1],
                                                     scale=1.0)
                                fc2_mms(fc, (0, 1, 2))
                        finbox["p"] = tc.alloc_tile_pool(name="fin", bufs=1)
                        epilogue(0)
                        epilogue(1)
                        epilogue(2)
                        # pass 2: fc2 for rc3 (w2 already prefetched)
                        psy[3] = ypsum.tile([P, 2, 512], F32, tag="y", name="psy3")
                        for fc in range(FC):
                            fc2_mms(fc, (3,))
                        epilogue(3)
                        finbox["p"].release()
                    w2_pool.release()

            w1_pool.release()


def _row_index(g):
    idx = np.empty(512, dtype=np.int64)
    r = 0
    for p in range(2):
        for s in range(2):
            j = 2 * p + s
            base = j * 512 + g * 128
            idx[r:r + 128] = np.arange(base, base + 128)
            r += 128
    return idx


def _causal_masks(g):
    kj = np.arange(P)[:, None]
    qi = np.arange(P)[None, :]
    m = np.empty((4, P, P), dtype=np.float32)
    for i in range(4):
        m[i] = np.where(kj <= qi + (g - i) * P, 1.0, 0.0)
    return m


def kernel(**inputs):
    if "nc" not in _CACHE:
        _CACHE["nc"] = _build()
    nc = _CACHE["nc"]

    bf = ml_dtypes.bfloat16
    x = np.asarray(inputs["x"], dtype=np.float32)
    w_bf = {k: np.ascontiguousarray(
        np.asarray(inputs[k], dtype=np.float32).astype(bf))
        for k in ("Wq", "Wk", "Wv", "Wo", "W1", "W2")}
    vecs = {k: np.ascontiguousarray(np.asarray(inputs[k], dtype=np.float32))
            for k in ("bq", "bk", "bv", "bo", "b1", "b2", "g1", "be1", "g2",
                      "be2")}

    in_maps = []
    for c in range(N_CORES):
        b, g = c // 4, c % 4
        idx = _row_index(g)
        xb = x[b]
        xrows = xb[idx]
        in_maps.append({
            "xT": np.ascontiguousarray(xb.T.astype(bf)),
            "xrT": np.ascontiguousarray(xrows.T.astype(bf)),
            "xr": np.ascontiguousarray(xrows),
            "wq": w_bf["Wq"], "wk": w_bf["Wk"], "wv": w_bf["Wv"],
            "wo": w_bf["Wo"], "w1": w_bf["W1"], "w2": w_bf["W2"],
            "bq": vecs["bq"], "bk": vecs["bk"],
            "bv": vecs["bv"].astype(bf), "bo": vecs["bo"].astype(bf),
            "b1": vecs["b1"], "b2": vecs["b2"].astype(bf),
            "g1": vecs["g1"].astype(bf), "be1": vecs["be1"].astype(bf),
            "g2": vecs["g2"].astype(bf), "be2": vecs["be2"].astype(bf),
            "cmask": _causal_masks(g).astype(bf),
        })

    res = run_bass_kernel_spmd(nc, in_maps, core_ids=list(range(N_CORES)))
    _CACHE["last_result"] = res

    outp = np.empty((B, L, D), dtype=np.float32)
    for c in range(N_CORES):
        b, g = c // 4, c % 4
        outp[b][_row_index(g)] = res.results[c]["out"].astype(np.float32)
    return outp


# revision 33
# speedup vs baseline: 1.0221x; 1.0082x over previous
"""Trainium2 Bass kernel for AttentionFFNBlock (B=2, L=2048, D=1024, H=16, FF=4096).

Sharding (8 cores, zero cross-core communication):
  core c -> batch b = c//4, group slot g = c%4.
  Each core owns 512 query rows of its batch, interleaved in 128-row blocks
  for causal load balance: global row = (2p+s)*512 + g*128 + i for local row
  r = p*256 + s*128 + i.  The core computes K/V for the full sequence
  (replicated inside the batch group), attention for its rows over all 16
  heads, then out-proj + LN1 + FFN + LN2 for its rows only.  Causality is
  enforced with per-core additive masks passed as input data (SPMD-safe).

Schedule: x arrives pre-transposed from the host (no DMA transposes); Q and
the first K/V chunks are projected up front; the remaining K / V projections
are interleaved into the attention head-pair loop so the PE stays dense
while the ACT engine works through the exp()s.  Scores skip dead (fully
masked) column blocks; head pairs share one exp instruction and alternate
PE row-groups (tile_position) so K=64 matmuls pack the array.  Wo/W1 are
prefetched as soon as SBUF frees up.  fc2 runs in two passes (rc pairs) so
the LN2 epilogues overlap the second pass's matmuls.

All matmuls in bf16 (fp32 PSUM accumulation); norms/softmax in fp32.
"""

import numpy as np
import ml_dtypes

import concourse.bass as bass
import concourse.mybir as mybir
import concourse.tile as tile
from concourse import bacc
from concourse.bass_utils import run_bass_kernel_spmd
from concourse.masks import make_identity

F32 = mybir.dt.float32
BF16 = mybir.dt.bfloat16
AF = mybir.ActivationFunctionType
ALU = mybir.AluOpType

N_CORES = 8
B, L, D = 2, 2048, 1024
H, HD = 16, 64
DFF = 4096
EPS = 1e-5
P = 128
NEG = -1e9

IC = D // P        # 8 contraction chunks of the model dim
TC = L // P        # 16 token chunks
FC = DFF // P      # 32 ff chunks
NPAIR = 8          # head pairs (= oc chunks)

_CACHE = {}


def _build():
    nc = bacc.Bacc("TRN2", target_bir_lowering=False, debug=False,
                   num_devices=N_CORES)

    def din(name, shape, dt=F32):
        return nc.dram_tensor(name, shape, dt, kind="ExternalInput").ap()

    io = dict(
        xT=din("xT", [D, L], BF16),               # x[b]^T (K/V source)
        xrT=din("xrT", [D, 512], BF16),           # owned rows^T (Q source)
        xr=din("xr", [512, D], F32),              # owned rows (residual)
        wq=din("wq", [D, D], BF16), wk=din("wk", [D, D], BF16),
        wv=din("wv", [D, D], BF16), wo=din("wo", [D, D], BF16),
        w1=din("w1", [D, DFF], BF16), w2=din("w2", [DFF, D], BF16),
        bq=din("bq", [D]), bk=din("bk", [D]), bv=din("bv", [D], BF16),
        bo=din("bo", [D], BF16), b1=din("b1", [DFF]), b2=din("b2", [D], BF16),
        g1=din("g1", [D], BF16), be1=din("be1", [D], BF16),
        g2=din("g2", [D], BF16), be2=din("be2", [D], BF16),
        cmask=din("cmask", [4, P, P], BF16),
        out=nc.dram_tensor("out", [512, D], BF16, kind="ExternalOutput").ap(),
    )

    with tile.TileContext(nc) as tc:
        _emit(nc, tc, io)
    nc.compile()
    return nc


def _layernorm(nc, pool, acc, eps_t, g_t, b_t, out_ap, g_eng=None,
               b_eng=None):
    """LayerNorm over the free axis (D=1024) of acc [128, 1024] -> out_ap."""
    stats = pool.tile([P, 2, 6], F32, tag="ln_stats")
    for sg in range(2):
        nc.vector.bn_stats(out=stats[:, sg, :], in_=acc[:, sg * 512:(sg + 1) * 512])
    mv = pool.tile([P, 2], F32, tag="ln_mv")
    nc.vector.bn_aggr(out=mv[:], in_=stats[:])
    rstd = pool.tile([P, 1], F32, tag="ln_rstd")
    nc.scalar.activation(out=rstd[:], in_=mv[:, 1:2], func=AF.Sqrt,
                         bias=eps_t[:], scale=1.0)
    nc.vector.reciprocal(out=rstd[:], in_=rstd[:])
    nmr = pool.tile([P, 1], F32, tag="ln_nmr")
    nc.vector.tensor_scalar(out=nmr[:], in0=mv[:, 0:1], scalar1=rstd[:],
                            scalar2=-1.0, op0=ALU.mult, op1=ALU.mult)
    u = pool.tile([P, D], BF16, tag="ln_u")
    nc.scalar.activation(out=u[:], in_=acc[:], func=AF.Identity,
                         bias=nmr[:], scale=rstd[:])
    (g_eng or nc.gpsimd).tensor_tensor(out=u[:], in0=u[:], in1=g_t[:, :],
                                       op=ALU.mult)
    (b_eng or nc.vector).tensor_tensor(out=out_ap, in0=u[:], in1=b_t[:, :],
                                       op=ALU.add)


def _emit(nc, tc, io):
    out = io["out"]

    with tc.tile_pool(name="const", bufs=1) as const:
        ao_pool = tc.alloc_tile_pool(name="ao_pool", bufs=1, side="right")
        # ---- constants / biases (tiles now; DMAs deferred past wk/xT) ----
        bq_t = const.tile([P, IC], F32)
        bk_t = const.tile([P, IC], F32)
        b1_t = const.tile([P, FC], F32)
        row_vecs = {}
        for nm in ("bv", "bo", "b2", "g1", "be1", "g2", "be2"):
            dt = F32 if nm.startswith("nope") else BF16
            rv = const.tile([P, D], dt, name=f"cv_{nm}")
            row_vecs[nm] = rv
        bv_t, bo_t, b2_t = row_vecs["bv"], row_vecs["bo"], row_vecs["b2"]
        g1_t, be1_t = row_vecs["g1"], row_vecs["be1"]
        g2_t, be2_t = row_vecs["g2"], row_vecs["be2"]
        cm_t = const.tile([P, 4, P], BF16)
        eps_t = const.tile([P, 1], F32)
        ident = const.tile([P, P], BF16)

        def early_dmas():
            nc.sync.dma_start(bq_t[:], io["bq"].rearrange("(o p) -> p o", p=P))
            nc.sync.dma_start(bk_t[:], io["bk"].rearrange("(o p) -> p o", p=P))
            nc.sync.dma_start(b1_t[:], io["b1"].rearrange("(f p) -> p f", p=P))
            nc.sync.dma_start(row_vecs["bv"][:],
                              io["bv"][None, :].to_broadcast([P, D]))
            nc.vector.memset(eps_t[:], EPS)

        def const_dmas():
            nc.sync.dma_start(cm_t[:], io["cmask"].rearrange("i p q -> p i q"))
            for nm in ("bo", "b2", "g1", "be1", "g2", "be2"):
                nc.sync.dma_start(row_vecs[nm][:],
                                  io[nm][None, :].to_broadcast([P, D]))
            make_identity(nc, ident[:])

        aoT = ao_pool.tile([P, IC, 512], BF16)   # attention output^T

        kv_pool = tc.alloc_tile_pool(name="kv_pool", bufs=1)
        ptile = tc.alloc_tile_pool(name="ptile", bufs=3)
        rtile = tc.alloc_tile_pool(name="rtile", bufs=2)
        spsum = tc.alloc_tile_pool(name="spsum", bufs=2, space="PSUM")
        avpsum = tc.alloc_tile_pool(name="avpsum", bufs=1, space="PSUM")
        if True:
            kT = kv_pool.tile([P, IC, L], BF16)
            v_all = kv_pool.tile([P, TC, H, HD + 1], BF16)
            qT = kv_pool.tile([P, IC, 512], BF16)
            nc.vector.memset(v_all[:, :, :, HD:], 1.0)

            proj_stream = []   # deferred (emit_mms, epilogue) generators

            def drain_proj(n):
                """Emit up to n deferred projection matmuls."""
                while n > 0 and proj_stream:
                    gen = proj_stream[0]
                    try:
                        next(gen)
                        n -= 1
                    except StopIteration:
                        proj_stream.pop(0)

            def attention(pair):
                oc = pair
                hA, hB = 2 * pair, 2 * pair + 1
                pavA = avpsum.tile([HD + 1, 512], F32, tag="avA")
                pavB = avpsum.tile([HD + 1, 512], F32, tag="avB")
                drain_proj(4)
                pts = []
                for kc in range(TC):
                    j0 = kc // 4
                    n0 = j0 * P
                    ps = spsum.tile([P, 2, 512], F32, tag="s")
                    nc.tensor.matmul(
                        ps[:, 0, n0:512],
                        kT[0:HD, oc, kc * P:(kc + 1) * P],
                        qT[0:HD, oc, n0:512], start=True, stop=True)
                    nc.tensor.matmul(
                        ps[:, 1, n0:512],
                        kT[HD:P, oc, kc * P:(kc + 1) * P],
                        qT[HD:P, oc, n0:512], start=True, stop=True)
                    pt = ptile.tile([P, 2, 512], BF16, tag="p")
                    nc.scalar.activation(out=pt[:, :, n0:512],
                                         in_=ps[:, :, n0:512],
                                         func=AF.Exp, scale=0.125)
                    # diagonal-window causal mask on block j0 (both heads):
                    # multiply by 0/1 post-exp (gpsimd cannot touch PSUM)
                    for j in range(2):
                        nc.gpsimd.tensor_tensor(
                            out=pt[:, j, n0:n0 + P], in0=pt[:, j, n0:n0 + P],
                            in1=cm_t[:, kc % 4, :], op=ALU.mult)
                    pts.append((kc, n0, pt))
                    drain_proj(4 if pair < 3 else 2)
                    # AV for the previous chunk (pipelined one deep)
                    if len(pts) >= 2:
                        pkc, pn0, ppt = pts.pop(0)
                        for j, (h, pav) in enumerate(((hA, pavA), (hB, pavB))):
                            nc.tensor.matmul(
                                pav[:, pn0:512], v_all[:, pkc, h, :],
                                ppt[:, j, pn0:512], start=(pkc == 0),
                                stop=False, skip_group_check=True)
                pkc, pn0, ppt = pts.pop(0)
                for j, (h, pav) in enumerate(((hA, pavA), (hB, pavB))):
                    nc.tensor.matmul(
                        pav[:, pn0:512], v_all[:, pkc, h, :],
                        ppt[:, j, pn0:512], start=False, stop=True,
                        skip_group_check=True)
                for hp, pav in ((0, pavA), (HD, pavB)):
                    rec = rtile.tile([1, 512], F32, tag="rec")
                    nc.vector.reciprocal(rec[:], pav[HD:HD + 1, :])
                    rec_b = rtile.tile([HD, 512], F32, tag="rec_b")
                    nc.gpsimd.partition_broadcast(rec_b[:], rec[0:1, :])
                    nc.vector.tensor_tensor(
                        out=aoT[hp:hp + HD, oc, :],
                        in0=pav[:HD, :], in1=rec_b[:], op=ALU.mult)

            # ---- projections (pairs 0..5 overlap with x_pool live) ----
            with (
                tc.tile_pool(name="x_pool", bufs=1) as x_pool,
                tc.tile_pool(name="ppsum", bufs=2, space="PSUM") as ppsum,
            ):
                wk_t = x_pool.tile([P, IC, D], BF16)
                xT_t = x_pool.tile([P, IC, L], BF16)
                wq_t = x_pool.tile([P, IC, D], BF16)
                xrT_t = x_pool.tile([P, IC, 512], BF16)
                wv_t = x_pool.tile([P, IC, D], BF16)
                wkr = io["wk"].rearrange("(i p) n -> p i n", p=P)
                wqr = io["wq"].rearrange("(i p) n -> p i n", p=P)
                wvr = io["wv"].rearrange("(i p) n -> p i n", p=P)
                xTr = io["xT"].rearrange("(i p) n -> p i n", p=P)
                nc.sync.dma_start(wk_t[:, :, 0:P], wkr[:, :, 0:P])
                nc.sync.dma_start(xT_t[:, 0:4, 0:512], xTr[:, 0:4, 0:512])
                nc.sync.dma_start(xT_t[:, 4:8, 0:512], xTr[:, 4:8, 0:512])
                nc.sync.dma_start(wq_t[:, :, 0:512], wqr[:, :, 0:512])
                nc.sync.dma_start(xrT_t[:],
                                  io["xrT"].rearrange("(i p) n -> p i n", p=P))
                nc.sync.dma_start(wv_t[:, :, 0:512], wvr[:, :, 0:512])
                early_dmas()
                nc.sync.dma_start(xT_t[:, :, 512:1024], xTr[:, :, 512:1024])
                nc.sync.dma_start(wk_t[:, :, P:512], wkr[:, :, P:512])
                nc.sync.dma_start(xT_t[:, :, 1024:1536], xTr[:, :, 1024:1536])
                nc.sync.dma_start(xT_t[:, :, 1536:2048], xTr[:, :, 1536:2048])
                nc.sync.dma_start(wk_t[:, :, 512:1024], wkr[:, :, 512:1024])
                nc.sync.dma_start(wq_t[:, :, 512:1024], wqr[:, :, 512:1024])
                const_dmas()
                nc.sync.dma_start(wv_t[:, :, 512:1024], wvr[:, :, 512:1024])

                def k_proj(oc):
                    for tcc in range(4):
                        ps = ppsum.tile([P, 512], F32, tag="proj")
                        for ic in range(IC):
                            nc.tensor.matmul(
                                ps[:], wk_t[:, ic, oc * P:(oc + 1) * P],
                                xT_t[:, ic, tcc * 512:(tcc + 1) * 512],
                                start=(ic == 0), stop=(ic == IC - 1))
                            yield
                        nc.vector.tensor_scalar_add(
                            out=kT[:, oc, tcc * 512:(tcc + 1) * 512],
                            in0=ps[:], scalar1=bk_t[:, oc:oc + 1])

                def q_proj(oc):
                    ps = ppsum.tile([P, 512], F32, tag="proj")
                    for ic in range(IC):
                        nc.tensor.matmul(
                            ps[:], wq_t[:, ic, oc * P:(oc + 1) * P],
                            xrT_t[:, ic, :],
                            start=(ic == 0), stop=(ic == IC - 1))
                        yield
                    nc.vector.tensor_scalar_add(
                        out=qT[:, oc, :], in0=ps[:], scalar1=bq_t[:, oc:oc + 1])

                def v_proj(tcc, hf):
                    ps = ppsum.tile([P, 512], F32, tag="proj")
                    for ic in range(IC):
                        nc.tensor.matmul(
                            ps[:], xT_t[:, ic, tcc * P:(tcc + 1) * P],
                            wv_t[:, ic, hf * 512:(hf + 1) * 512],
                            start=(ic == 0), stop=(ic == IC - 1))
                        yield
                    nc.vector.tensor_tensor(
                        out=v_all[:, tcc, hf * 8:(hf + 1) * 8, :HD],
                        in0=ps.rearrange("p (h d) -> p h d", d=HD),
                        in1=bv_t[:, hf * 512:(hf + 1) * 512]
                        .rearrange("p (h d) -> p h d", d=HD),
                        op=ALU.add)

                # upfront, ordered to match serial DMA arrival
                def adv(gen, n):
                    for _ in range(n):
                        try:
                            next(gen)
                        except StopIteration:
                            return
                k0, k1 = k_proj(0), k_proj(1)
                qs = [q_proj(oc) for oc in range(IC)]
                v0s = [v_proj(tcc, 0) for tcc in range(TC)]
                adv(k0, 8)                       # K0.tcc0 (wk0+xT0)
                for oc in range(4):
                    adv(qs[oc], 9)               # Q0-3 (wq0+xrT)
                for tcc in range(4):
                    adv(v0s[tcc], 9)             # V0 tcc0-3 (wv0+xT0)
                adv(k0, 100)                     # K0 rest (xT1-3)
                adv(k1, 32)                      # K1 (wk1)
                for oc in range(4, IC):
                    adv(qs[oc], 9)               # Q4-7 (wq1)
                for tcc in range(4, TC):
                    adv(v0s[tcc], 9)             # V0 rest
                for g in [k0, k1] + qs + v0s:
                    adv(g, 100)
                # deferred: K2,K3, all of V1, K4..K7 — drained inside attention
                proj_stream.extend([k_proj(2), k_proj(3)])
                proj_stream.extend(v_proj(tcc, 1) for tcc in range(TC))
                proj_stream.extend(k_proj(oc) for oc in range(4, IC))

                for pair in range(7):
                    attention(pair)
                drain_proj(1 << 30)

            # x_pool freed: prefetch xr + wo under attn 7 (right side)
            xrr_pool = tc.alloc_tile_pool(name="xrr_pool", bufs=1, side="right")
            xr_nat = xrr_pool.tile([P, 4, D], F32)
            nc.sync.dma_start(xr_nat[:],
                              io["xr"].rearrange("(rc p) d -> p rc d", p=P))
            wo_pool = tc.alloc_tile_pool(name="wo_pool", bufs=1, side="right")
            wo_t = wo_pool.tile([P, IC, D], BF16)
            wor = io["wo"].rearrange("(i p) n -> p i n", p=P)
            for h2 in range(2):
                nc.sync.dma_start(wo_t[:, :, h2 * 512:(h2 + 1) * 512],
                                  wor[:, :, h2 * 512:(h2 + 1) * 512])

            attention(7)

            # free the attention pools (non-LIFO: wo/w1a stay live)
            avpsum.release()
            spsum.release()
            rtile.release()
            ptile.release()
            kv_pool.release()

            w1_pool = tc.alloc_tile_pool(name="w1_pool", bufs=1)
            w1_t = w1_pool.tile([P, IC, DFF], BF16)
            w1r = io["w1"].rearrange("(i p) n -> p i n", p=P)
            for c in range(8):
                nc.sync.dma_start(w1_t[:, :, c * 512:(c + 1) * 512],
                                  w1r[:, :, c * 512:(c + 1) * 512])


            if True:
                # ---- out-proj + LN1 + transpose ----
                with tc.tile_pool(name="t_pool", bufs=1) as t_pool:
                    tbf = t_pool.tile([P, 4, D], BF16)    # LN1 out (residual)
                    tT = t_pool.tile([P, IC, 512], BF16)  # LN1 out transposed

                    with (
                        tc.tile_pool(name="lnt", bufs=4) as lnt,
                        tc.tile_pool(name="opsum", bufs=4, space="PSUM") as opsum,
                        tc.tile_pool(name="trpsum", bufs=4, space="PSUM") as trpsum,
                    ):
                        for rc in range(4):
                            acc = lnt.tile([P, D], F32, tag="acc")
                            for n2 in range(2):
                                ps = opsum.tile([P, 512], F32, tag="o")
                                for dc in range(IC):
                                    nc.tensor.matmul(
                                        ps[:], aoT[:, dc, rc * P:(rc + 1) * P],
                                        wo_t[:, dc, n2 * 512:(n2 + 1) * 512],
                                        start=(dc == 0), stop=(dc == IC - 1))
                                nc.vector.tensor_tensor(
                                    out=acc[:, n2 * 512:(n2 + 1) * 512],
                                    in0=ps[:],
                                    in1=xr_nat[:, rc, n2 * 512:(n2 + 1) * 512],
                                    op=ALU.add)
                            nc.vector.tensor_tensor(
                                out=acc[:], in0=acc[:], in1=bo_t[:, :],
                                op=ALU.add)
                            _layernorm(nc, lnt, acc, eps_t, g1_t, be1_t,
                                       tbf[:, rc, :])
                        for rc in range(4):
                            for ic in range(IC):
                                pst = trpsum.tile([P, P], BF16, tag="tr")
                                nc.tensor.transpose(
                                    pst[:], tbf[:, rc, ic * P:(ic + 1) * P],
                                    ident[:])
                                nc.scalar.copy(
                                    tT[:, ic, rc * P:(rc + 1) * P], pst[:])

                    wo_pool.release()
                    xrr_pool.release()
                    ao_pool.release()

                    # ================= FFN =================
                    w2_pool = tc.alloc_tile_pool(name="w2_pool", bufs=1)
                    w2_t = w2_pool.tile([P, FC, D], BF16)
                    w2r = io["w2"].rearrange("(f p) n -> p f n", p=P)
                    for grp in range(8):
                        nc.sync.dma_start(w2_t[:, grp * 4:(grp + 1) * 4, :],
                                          w2r[:, grp * 4:(grp + 1) * 4, :])
                    with (
                        tc.tile_pool(name="h_pool", bufs=1) as h_pool,
                        tc.tile_pool(name="fpsum", bufs=2, space="PSUM") as fpsum,
                        tc.tile_pool(name="ypsum", bufs=3, space="PSUM") as ypsum,
                    ):
                        hT = h_pool.tile([P, FC, 512], BF16)
                        psy = {}

                        def fc2_mms(fc, rcs):
                            for rc in rcs:
                                for n2 in range(2):
                                    nc.tensor.matmul(
                                        psy[rc][:, n2, :],
                                        hT[:, fc, rc * P:(rc + 1) * P],
                                        w2_t[:, fc, n2 * 512:(n2 + 1) * 512],
                                        start=(fc == 0), stop=(fc == FC - 1))

                        finbox = {}

                        def epilogue(rc):
                            fin = finbox["p"]
                            acc = fin.tile([P, D], F32, tag="acc2", bufs=2)
                            for n2 in range(2):
                                nc.vector.tensor_tensor(
                                    out=acc[:, n2 * 512:(n2 + 1) * 512],
                                    in0=psy[rc][:, n2, :],
                                    in1=tbf[:, rc, n2 * 512:(n2 + 1) * 512],
                                    op=ALU.add)
                            nc.vector.tensor_tensor(
                                out=acc[:], in0=acc[:], in1=b2_t[:, :],
                                op=ALU.add)
                            res = fin.tile([P, D], BF16, tag="res", bufs=2)
                            _layernorm(nc, fin, acc, eps_t, g2_t, be2_t,
                                       res[:], g_eng=nc.vector,
                                       b_eng=nc.vector)
                            nc.sync.dma_start(
                                out.rearrange("(rc p) d -> p rc d", p=P)[:, rc, :],
                                res[:])

                        # pass 1: fc1 + fc2 for rc 0,1,2 interleaved per fc
                        psy[0] = ypsum.tile([P, 2, 512], F32, tag="y", name="psy0")
                        psy[1] = ypsum.tile([P, 2, 512], F32, tag="y", name="psy1")
                        psy[2] = ypsum.tile([P, 2, 512], F32, tag="y", name="psy2")
                        for grp in range(8):
                            for k in range(4):
                                fc = grp * 4 + k
                                ps = fpsum.tile([P, 512], F32, tag="f1")
                                for ic in range(IC):
                                    nc.tensor.matmul(
                                        ps[:],
                                        w1_t[:, ic, fc * P:(fc + 1) * P],
                                        tT[:, ic, :],
                                        start=(ic == 0), stop=(ic == IC - 1))
                                nc.scalar.activation(out=hT[:, fc, :], in_=ps[:],
                                                     func=AF.Gelu,
                                                     bias=b1_t[:, fc:fc + 1],
                                                     scale=1.0)
                                fc2_mms(fc, (0, 1, 2))
                        finbox["p"] = tc.alloc_tile_pool(name="fin", bufs=1)
                        epilogue(0)
                        epilogue(1)
                        epilogue(2)
                        # pass 2: fc2 for rc3 (w2 already prefetched)
                        psy[3] = ypsum.tile([P, 2, 512], F32, tag="y", name="psy3")
                        for fc in range(FC):
                            fc2_mms(fc, (3,))
                        epilogue(3)
                        finbox["p"].release()
                    w2_pool.release()

            w1_pool.release()


def _row_index(g):
    idx = np.empty(512, dtype=np.int64)
    r = 0
    for p in range(2):
        for s in range(2):
            j = 2 * p + s
            base = j * 512 + g * 128
            idx[r:r + 128] = np.arange(base, base + 128)
            r += 128
    return idx


def _causal_masks(g):
    kj = np.arange(P)[:, None]
    qi = np.arange(P)[None, :]
    m = np.empty((4, P, P), dtype=np.float32)
    for i in range(4):
        m[i] = np.where(kj <= qi + (g - i) * P, 1.0, 0.0)
    return m


def kernel(**inputs):
    if "nc" not in _CACHE:
        _CACHE["nc"] = _build()
    nc = _CACHE["nc"]

    bf = ml_dtypes.bfloat16
    x = np.asarray(inputs["x"], dtype=np.float32)
    w_bf = {k: np.ascontiguousarray(
        np.asarray(inputs[k], dtype=np.float32).astype(bf))
        for k in ("Wq", "Wk", "Wv", "Wo", "W1", "W2")}
    vecs = {k: np.ascontiguousarray(np.asarray(inputs[k], dtype=np.float32))
            for k in ("bq", "bk", "bv", "bo", "b1", "b2", "g1", "be1", "g2",
                      "be2")}

    in_maps = []
    for c in range(N_CORES):
        b, g = c // 4, c % 4
        idx = _row_index(g)
        xb = x[b]
        xrows = xb[idx]
        in_maps.append({
            "xT": np.ascontiguousarray(xb.T.astype(bf)),
            "xrT": np.ascontiguousarray(xrows.T.astype(bf)),
            "xr": np.ascontiguousarray(xrows),
            "wq": w_bf["Wq"], "wk": w_bf["Wk"], "wv": w_bf["Wv"],
            "wo": w_bf["Wo"], "w1": w_bf["W1"], "w2": w_bf["W2"],
            "bq": vecs["bq"], "bk": vecs["bk"],
            "bv": vecs["bv"].astype(bf), "bo": vecs["bo"].astype(bf),
            "b1": vecs["b1"], "b2": vecs["b2"].astype(bf),
            "g1": vecs["g1"].astype(bf), "be1": vecs["be1"].astype(bf),
            "g2": vecs["g2"].astype(bf), "be2": vecs["be2"].astype(bf),
            "cmask": _causal_masks(g).astype(bf),
        })

    res = run_bass_kernel_spmd(nc, in_maps, core_ids=list(range(N_CORES)))
    _CACHE["last_result"] = res

    outp = np.empty((B, L, D), dtype=np.float32)
    for c in range(N_CORES):
        b, g = c // 4, c % 4
        outp[b][_row_index(g)] = res.results[c]["out"].astype(np.float32)
    return outp


# revision 35
# speedup vs baseline: 1.0514x; 1.0287x over previous
"""Trainium2 Bass kernel for AttentionFFNBlock (B=2, L=2048, D=1024, H=16, FF=4096).

Sharding (8 cores, zero cross-core communication):
  core c -> batch b = c//4, group slot g = c%4.
  Each core owns 512 query rows of its batch, interleaved in 128-row blocks
  for causal load balance: global row = (2p+s)*512 + g*128 + i for local row
  r = p*256 + s*128 + i.  The core computes K/V for the full sequence
  (replicated inside the batch group), attention for its rows over all 16
  heads, then out-proj + LN1 + FFN + LN2 for its rows only.  Causality is
  enforced with per-core additive masks passed as input data (SPMD-safe).

Schedule: x arrives pre-transposed from the host (no DMA transposes); Q and
the first K/V chunks are projected up front; the remaining K / V projections
are interleaved into the attention head-pair loop so the PE stays dense
while the ACT engine works through the exp()s.  Scores skip dead (fully
masked) column blocks; head pairs share one exp instruction and alternate
PE row-groups (tile_position) so K=64 matmuls pack the array.  Wo/W1 are
prefetched as soon as SBUF frees up.  fc2 runs in two passes (rc pairs) so
the LN2 epilogues overlap the second pass's matmuls.

All matmuls in bf16 (fp32 PSUM accumulation); norms/softmax in fp32.
"""

import numpy as np
import ml_dtypes

import concourse.bass as bass
import concourse.mybir as mybir
import concourse.tile as tile
from concourse import bacc
from concourse.bass_utils import run_bass_kernel_spmd
from concourse.masks import make_identity

F32 = mybir.dt.float32
BF16 = mybir.dt.bfloat16
AF = mybir.ActivationFunctionType
ALU = mybir.AluOpType

N_CORES = 8
B, L, D = 2, 2048, 1024
H, HD = 16, 64
DFF = 4096
EPS = 1e-5
P = 128
NEG = -1e9

IC = D // P        # 8 contraction chunks of the model dim
TC = L // P        # 16 token chunks
FC = DFF // P      # 32 ff chunks
NPAIR = 8          # head pairs (= oc chunks)

_CACHE = {}


def _build():
    nc = bacc.Bacc("TRN2", target_bir_lowering=False, debug=False,
                   num_devices=N_CORES)

    def din(name, shape, dt=F32):
        return nc.dram_tensor(name, shape, dt, kind="ExternalInput").ap()

    io = dict(
        xT=din("xT", [D, L], BF16),               # x[b]^T (K/V source)
        xrT=din("xrT", [D, 512], BF16),           # owned rows^T (Q source)
        xr=din("xr", [512, D], F32),              # owned rows (residual)
        wq=din("wq", [D, D], BF16), wk=din("wk", [D, D], BF16),
        wv=din("wv", [D, D], BF16), wo=din("wo", [D, D], BF16),
        w1=din("w1", [D, DFF], BF16), w2=din("w2", [DFF, D], BF16),
        bq=din("bq", [D]), bk=din("bk", [D]), bv=din("bv", [D], BF16),
        bo=din("bo", [D], BF16), b1=din("b1", [DFF]), b2=din("b2", [D], BF16),
        g1=din("g1", [D], BF16), be1=din("be1", [D], BF16),
        g2=din("g2", [D], BF16), be2=din("be2", [D], BF16),
        cmask=din("cmask", [4, P, P], BF16),
        out=nc.dram_tensor("out", [512, D], BF16, kind="ExternalOutput").ap(),
    )

    with tile.TileContext(nc) as tc:
        _emit(nc, tc, io)
    nc.compile()
    return nc


def _layernorm(nc, pool, acc, eps_t, g_t, b_t, out_ap, g_eng=None,
               b_eng=None):
    """LayerNorm over the free axis (D=1024) of acc [128, 1024] -> out_ap."""
    stats = pool.tile([P, 2, 6], F32, tag="ln_stats")
    for sg in range(2):
        nc.vector.bn_stats(out=stats[:, sg, :], in_=acc[:, sg * 512:(sg + 1) * 512])
    mv = pool.tile([P, 2], F32, tag="ln_mv")
    nc.vector.bn_aggr(out=mv[:], in_=stats[:])
    rstd = pool.tile([P, 1], F32, tag="ln_rstd")
    nc.scalar.activation(out=rstd[:], in_=mv[:, 1:2], func=AF.Sqrt,
                         bias=eps_t[:], scale=1.0)
    nc.vector.reciprocal(out=rstd[:], in_=rstd[:])
    nmr = pool.tile([P, 1], F32, tag="ln_nmr")
    nc.vector.tensor_scalar(out=nmr[:], in0=mv[:, 0:1], scalar1=rstd[:],
                            scalar2=-1.0, op0=ALU.mult, op1=ALU.mult)
    u = pool.tile([P, D], BF16, tag="ln_u")
    nc.scalar.activation(out=u[:], in_=acc[:], func=AF.Identity,
                         bias=nmr[:], scale=rstd[:])
    (g_eng or nc.gpsimd).tensor_tensor(out=u[:], in0=u[:], in1=g_t[:, :],
                                       op=ALU.mult)
    (b_eng or nc.vector).tensor_tensor(out=out_ap, in0=u[:], in1=b_t[:, :],
                                       op=ALU.add)


def _emit(nc, tc, io):
    out = io["out"]

    with tc.tile_pool(name="const", bufs=1) as const:
        ao_pool = tc.alloc_tile_pool(name="ao_pool", bufs=1, side="right")
        # ---- constants / biases (tiles now; DMAs deferred past wk/xT) ----
        bq_t = const.tile([P, IC], F32)
        bk_t = const.tile([P, IC], F32)
        b1_t = const.tile([P, FC], F32)
        row_vecs = {}
        for nm in ("bv", "bo", "b2", "g1", "be1", "g2", "be2"):
            dt = F32 if nm.startswith("nope") else BF16
            rv = const.tile([P, D], dt, name=f"cv_{nm}")
            row_vecs[nm] = rv
        bv_t, bo_t, b2_t = row_vecs["bv"], row_vecs["bo"], row_vecs["b2"]
        g1_t, be1_t = row_vecs["g1"], row_vecs["be1"]
        g2_t, be2_t = row_vecs["g2"], row_vecs["be2"]
        cm_t = const.tile([P, 4, P], BF16)
        eps_t = const.tile([P, 1], F32)
        ident = const.tile([P, P], BF16)

        def early_dmas():
            nc.sync.dma_start(bq_t[:], io["bq"].rearrange("(o p) -> p o", p=P))
            nc.sync.dma_start(bk_t[:], io["bk"].rearrange("(o p) -> p o", p=P))
            nc.sync.dma_start(b1_t[:], io["b1"].rearrange("(f p) -> p f", p=P))
            nc.sync.dma_start(row_vecs["bv"][:],
                              io["bv"][None, :].to_broadcast([P, D]))
            nc.vector.memset(eps_t[:], EPS)

        def const_dmas():
            nc.sync.dma_start(cm_t[:], io["cmask"].rearrange("i p q -> p i q"))
            for nm in ("bo", "b2", "g1", "be1", "g2", "be2"):
                nc.sync.dma_start(row_vecs[nm][:],
                                  io[nm][None, :].to_broadcast([P, D]))
            make_identity(nc, ident[:])

        aoT = ao_pool.tile([P, IC, 512], BF16)   # attention output^T

        kv_pool = tc.alloc_tile_pool(name="kv_pool", bufs=1)
        ptile = tc.alloc_tile_pool(name="ptile", bufs=3)
        rtile = tc.alloc_tile_pool(name="rtile", bufs=2)
        spsum = tc.alloc_tile_pool(name="spsum", bufs=2, space="PSUM")
        avpsum = tc.alloc_tile_pool(name="avpsum", bufs=1, space="PSUM")
        if True:
            kT = kv_pool.tile([P, IC, L], BF16)
            v_all = kv_pool.tile([P, TC, H, HD + 1], BF16)
            qT = kv_pool.tile([P, IC, 512], BF16)
            nc.vector.memset(v_all[:, :, :, HD:], 1.0)

            proj_stream = []   # deferred (emit_mms, epilogue) generators

            def drain_proj(n):
                """Emit up to n deferred projection matmuls."""
                while n > 0 and proj_stream:
                    gen = proj_stream[0]
                    try:
                        next(gen)
                        n -= 1
                    except StopIteration:
                        proj_stream.pop(0)

            def attention(pair, prev_epi=None):
                oc = pair
                hA, hB = 2 * pair, 2 * pair + 1
                pavA = avpsum.tile([HD + 1, 512], F32, tag="avA")
                pavB = avpsum.tile([HD + 1, 512], F32, tag="avB")
                drain_proj(4)
                pts = []
                for kc in range(TC):
                    if kc == 2 and prev_epi is not None:
                        prev_epi()
                        prev_epi = None
                    j0 = kc // 4
                    n0 = j0 * P
                    ps = spsum.tile([P, 2, 512], F32, tag="s")
                    nc.tensor.matmul(
                        ps[:, 0, n0:512],
                        kT[0:HD, oc, kc * P:(kc + 1) * P],
                        qT[0:HD, oc, n0:512], start=True, stop=True)
                    nc.tensor.matmul(
                        ps[:, 1, n0:512],
                        kT[HD:P, oc, kc * P:(kc + 1) * P],
                        qT[HD:P, oc, n0:512], start=True, stop=True)
                    pt = ptile.tile([P, 2, 512], BF16, tag="p")
                    nc.scalar.activation(out=pt[:, :, n0:512],
                                         in_=ps[:, :, n0:512],
                                         func=AF.Exp, scale=0.125)
                    # diagonal-window causal mask on block j0 (both heads):
                    # multiply by 0/1 post-exp (gpsimd cannot touch PSUM)
                    for j in range(2):
                        nc.gpsimd.tensor_tensor(
                            out=pt[:, j, n0:n0 + P], in0=pt[:, j, n0:n0 + P],
                            in1=cm_t[:, kc % 4, :], op=ALU.mult)
                    pts.append((kc, n0, pt))
                    drain_proj(4 if pair < 3 else 2)
                    # AV lagging two chunks so the exp/mask chain never
                    # stalls the in-order PE queue
                    if len(pts) >= 3:
                        pkc, pn0, ppt = pts.pop(0)
                        for j, (h, pav) in enumerate(((hA, pavA), (hB, pavB))):
                            nc.tensor.matmul(
                                pav[:, pn0:512], v_all[:, pkc, h, :],
                                ppt[:, j, pn0:512], start=(pkc == 0),
                                stop=False, skip_group_check=True)
                while pts:
                    pkc, pn0, ppt = pts.pop(0)
                    last = not pts
                    for j, (h, pav) in enumerate(((hA, pavA), (hB, pavB))):
                        nc.tensor.matmul(
                            pav[:, pn0:512], v_all[:, pkc, h, :],
                            ppt[:, j, pn0:512], start=(pkc == 0),
                            stop=last, skip_group_check=True)
                def epi():
                    for hp, pav in ((0, pavA), (HD, pavB)):
                        rec = rtile.tile([1, 512], F32, tag="rec")
                        nc.vector.reciprocal(rec[:], pav[HD:HD + 1, :])
                        rec_b = rtile.tile([HD, 512], F32, tag="rec_b")
                        nc.gpsimd.partition_broadcast(rec_b[:], rec[0:1, :])
                        nc.vector.tensor_tensor(
                            out=aoT[hp:hp + HD, oc, :],
                            in0=pav[:HD, :], in1=rec_b[:], op=ALU.mult)
                return epi

            # ---- projections (pairs 0..5 overlap with x_pool live) ----
            with (
                tc.tile_pool(name="x_pool", bufs=1) as x_pool,
                tc.tile_pool(name="ppsum", bufs=2, space="PSUM") as ppsum,
            ):
                wk_t = x_pool.tile([P, IC, D], BF16)
                xT_t = x_pool.tile([P, IC, L], BF16)
                wq_t = x_pool.tile([P, IC, D], BF16)
                xrT_t = x_pool.tile([P, IC, 512], BF16)
                wv_t = x_pool.tile([P, IC, D], BF16)
                wkr = io["wk"].rearrange("(i p) n -> p i n", p=P)
                wqr = io["wq"].rearrange("(i p) n -> p i n", p=P)
                wvr = io["wv"].rearrange("(i p) n -> p i n", p=P)
                xTr = io["xT"].rearrange("(i p) n -> p i n", p=P)
                nc.sync.dma_start(wk_t[:, :, 0:P], wkr[:, :, 0:P])
                nc.sync.dma_start(xT_t[:, 0:4, 0:512], xTr[:, 0:4, 0:512])
                nc.sync.dma_start(xT_t[:, 4:8, 0:512], xTr[:, 4:8, 0:512])
                nc.sync.dma_start(wq_t[:, :, 0:512], wqr[:, :, 0:512])
                nc.sync.dma_start(xrT_t[:],
                                  io["xrT"].rearrange("(i p) n -> p i n", p=P))
                nc.sync.dma_start(wv_t[:, :, 0:512], wvr[:, :, 0:512])
                early_dmas()
                nc.sync.dma_start(xT_t[:, :, 512:1024], xTr[:, :, 512:1024])
                nc.sync.dma_start(wk_t[:, :, P:512], wkr[:, :, P:512])
                nc.sync.dma_start(xT_t[:, :, 1024:1536], xTr[:, :, 1024:1536])
                nc.sync.dma_start(xT_t[:, :, 1536:2048], xTr[:, :, 1536:2048])
                nc.sync.dma_start(wk_t[:, :, 512:1024], wkr[:, :, 512:1024])
                nc.sync.dma_start(wq_t[:, :, 512:1024], wqr[:, :, 512:1024])
                const_dmas()
                nc.sync.dma_start(wv_t[:, :, 512:1024], wvr[:, :, 512:1024])

                def k_proj(oc):
                    for tcc in range(4):
                        ps = ppsum.tile([P, 512], F32, tag="proj")
                        for ic in range(IC):
                            nc.tensor.matmul(
                                ps[:], wk_t[:, ic, oc * P:(oc + 1) * P],
                                xT_t[:, ic, tcc * 512:(tcc + 1) * 512],
                                start=(ic == 0), stop=(ic == IC - 1))
                            yield
                        nc.vector.tensor_scalar_add(
                            out=kT[:, oc, tcc * 512:(tcc + 1) * 512],
                            in0=ps[:], scalar1=bk_t[:, oc:oc + 1])

                def q_proj(oc):
                    ps = ppsum.tile([P, 512], F32, tag="proj")
                    for ic in range(IC):
                        nc.tensor.matmul(
                            ps[:], wq_t[:, ic, oc * P:(oc + 1) * P],
                            xrT_t[:, ic, :],
                            start=(ic == 0), stop=(ic == IC - 1))
                        yield
                    nc.vector.tensor_scalar_add(
                        out=qT[:, oc, :], in0=ps[:], scalar1=bq_t[:, oc:oc + 1])

                def v_proj(tcc, hf):
                    ps = ppsum.tile([P, 512], F32, tag="proj")
                    for ic in range(IC):
                        nc.tensor.matmul(
                            ps[:], xT_t[:, ic, tcc * P:(tcc + 1) * P],
                            wv_t[:, ic, hf * 512:(hf + 1) * 512],
                            start=(ic == 0), stop=(ic == IC - 1))
                        yield
                    nc.vector.tensor_tensor(
                        out=v_all[:, tcc, hf * 8:(hf + 1) * 8, :HD],
                        in0=ps.rearrange("p (h d) -> p h d", d=HD),
                        in1=bv_t[:, hf * 512:(hf + 1) * 512]
                        .rearrange("p (h d) -> p h d", d=HD),
                        op=ALU.add)

                # upfront, ordered to match serial DMA arrival
                def adv(gen, n):
                    for _ in range(n):
                        try:
                            next(gen)
                        except StopIteration:
                            return
                k0, k1 = k_proj(0), k_proj(1)
                qs = [q_proj(oc) for oc in range(IC)]
                v0s = [v_proj(tcc, 0) for tcc in range(TC)]
                adv(k0, 8)                       # K0.tcc0 (wk0+xT0)
                for oc in range(4):
                    adv(qs[oc], 9)               # Q0-3 (wq0+xrT)
                for tcc in range(4):
                    adv(v0s[tcc], 9)             # V0 tcc0-3 (wv0+xT0)
                adv(k0, 100)                     # K0 rest (xT1-3)
                adv(k1, 32)                      # K1 (wk1)
                for oc in range(4, IC):
                    adv(qs[oc], 9)               # Q4-7 (wq1)
                for tcc in range(4, TC):
                    adv(v0s[tcc], 9)             # V0 rest
                for g in [k0, k1] + qs + v0s:
                    adv(g, 100)
                # deferred: K2,K3, all of V1, K4..K7 — drained inside attention
                proj_stream.extend([k_proj(2), k_proj(3)])
                proj_stream.extend(v_proj(tcc, 1) for tcc in range(TC))
                proj_stream.extend(k_proj(oc) for oc in range(4, IC))

                prev_epi = None
                for pair in range(7):
                    prev_epi = attention(pair, prev_epi)
                drain_proj(1 << 30)

            # x_pool freed: prefetch xr + wo under attn 7 (right side)
            xrr_pool = tc.alloc_tile_pool(name="xrr_pool", bufs=1, side="right")
            xr_nat = xrr_pool.tile([P, 4, D], F32)
            nc.sync.dma_start(xr_nat[:],
                              io["xr"].rearrange("(rc p) d -> p rc d", p=P))
            wo_pool = tc.alloc_tile_pool(name="wo_pool", bufs=1, side="right")
            wo_t = wo_pool.tile([P, IC, D], BF16)
            wor = io["wo"].rearrange("(i p) n -> p i n", p=P)
            for h2 in range(2):
                nc.sync.dma_start(wo_t[:, :, h2 * 512:(h2 + 1) * 512],
                                  wor[:, :, h2 * 512:(h2 + 1) * 512])

            prev_epi = attention(7, prev_epi)
            prev_epi()

            # free the attention pools (non-LIFO: wo/w1a stay live)
            avpsum.release()
            spsum.release()
            rtile.release()
            ptile.release()
            kv_pool.release()

            w1_pool = tc.alloc_tile_pool(name="w1_pool", bufs=1)
            w1_t = w1_pool.tile([P, IC, DFF], BF16)
            w1r = io["w1"].rearrange("(i p) n -> p i n", p=P)
            for c in range(8):
                nc.sync.dma_start(w1_t[:, :, c * 512:(c + 1) * 512],
                                  w1r[:, :, c * 512:(c + 1) * 512])


            if True:
                # ---- out-proj + LN1 + transpose ----
                with tc.tile_pool(name="t_pool", bufs=1) as t_pool:
                    tbf = t_pool.tile([P, 4, D], BF16)    # LN1 out (residual)
                    tT = t_pool.tile([P, IC, 512], BF16)  # LN1 out transposed

                    with (
                        tc.tile_pool(name="lnt", bufs=4) as lnt,
                        tc.tile_pool(name="opsum", bufs=4, space="PSUM") as opsum,
                        tc.tile_pool(name="trpsum", bufs=4, space="PSUM") as trpsum,
                    ):
                        for rc in range(4):
                            acc = lnt.tile([P, D], F32, tag="acc")
                            for n2 in range(2):
                                ps = opsum.tile([P, 512], F32, tag="o")
                                for dc in range(IC):
                                    nc.tensor.matmul(
                                        ps[:], aoT[:, dc, rc * P:(rc + 1) * P],
                                        wo_t[:, dc, n2 * 512:(n2 + 1) * 512],
                                        start=(dc == 0), stop=(dc == IC - 1))
                                nc.vector.tensor_tensor(
                                    out=acc[:, n2 * 512:(n2 + 1) * 512],
                                    in0=ps[:],
                                    in1=xr_nat[:, rc, n2 * 512:(n2 + 1) * 512],
                                    op=ALU.add)
                            nc.vector.tensor_tensor(
                                out=acc[:], in0=acc[:], in1=bo_t[:, :],
                                op=ALU.add)
                            _layernorm(nc, lnt, acc, eps_t, g1_t, be1_t,
                                       tbf[:, rc, :])
                        for rc in range(4):
                            for ic in range(IC):
                                pst = trpsum.tile([P, P], BF16, tag="tr")
                                nc.tensor.transpose(
                                    pst[:], tbf[:, rc, ic * P:(ic + 1) * P],
                                    ident[:])
                                nc.scalar.copy(
                                    tT[:, ic, rc * P:(rc + 1) * P], pst[:])

                    wo_pool.release()
                    xrr_pool.release()
                    ao_pool.release()

                    # ================= FFN =================
                    w2_pool = tc.alloc_tile_pool(name="w2_pool", bufs=1)
                    w2_t = w2_pool.tile([P, FC, D], BF16)
                    w2r = io["w2"].rearrange("(f p) n -> p f n", p=P)
                    for grp in range(8):
                        nc.sync.dma_start(w2_t[:, grp * 4:(grp + 1) * 4, :],
                                          w2r[:, grp * 4:(grp + 1) * 4, :])
                    with (
                        tc.tile_pool(name="h_pool", bufs=1) as h_pool,
                        tc.tile_pool(name="fpsum", bufs=2, space="PSUM") as fpsum,
                        tc.tile_pool(name="ypsum", bufs=3, space="PSUM") as ypsum,
                    ):
                        hT = h_pool.tile([P, FC, 512], BF16)
                        psy = {}

                        def fc2_mms(fc, rcs):
                            for rc in rcs:
                                for n2 in range(2):
                                    nc.tensor.matmul(
                                        psy[rc][:, n2, :],
                                        hT[:, fc, rc * P:(rc + 1) * P],
                                        w2_t[:, fc, n2 * 512:(n2 + 1) * 512],
                                        start=(fc == 0), stop=(fc == FC - 1))

                        finbox = {}

                        def epilogue(rc):
                            fin = finbox["p"]
                            acc = fin.tile([P, D], F32, tag="acc2", bufs=2)
                            for n2 in range(2):
                                nc.vector.tensor_tensor(
                                    out=acc[:, n2 * 512:(n2 + 1) * 512],
                                    in0=psy[rc][:, n2, :],
                                    in1=tbf[:, rc, n2 * 512:(n2 + 1) * 512],
                                    op=ALU.add)
                            nc.vector.tensor_tensor(
                                out=acc[:], in0=acc[:], in1=b2_t[:, :],
                                op=ALU.add)
                            res = fin.tile([P, D], BF16, tag="res", bufs=2)
                            _layernorm(nc, fin, acc, eps_t, g2_t, be2_t,
                                       res[:], g_eng=nc.vector,
                                       b_eng=nc.vector)
                            nc.sync.dma_start(
                                out.rearrange("(rc p) d -> p rc d", p=P)[:, rc, :],
                                res[:])

                        # pass 1: fc1 + fc2 for rc 0,1,2 interleaved per fc
                        psy[0] = ypsum.tile([P, 2, 512], F32, tag="y", name="psy0")
                        psy[1] = ypsum.tile([P, 2, 512], F32, tag="y", name="psy1")
                        psy[2] = ypsum.tile([P, 2, 512], F32, tag="y", name="psy2")
                        for grp in range(8):
                            for k in range(4):
                                fc = grp * 4 + k
                                ps = fpsum.tile([P, 512], F32, tag="f1")
                                for ic in range(IC):
                                    nc.tensor.matmul(
                                        ps[:],
                                        w1_t[:, ic, fc * P:(fc + 1) * P],
                                        tT[:, ic, :],
                                        start=(ic == 0), stop=(ic == IC - 1))
                                nc.scalar.activation(out=hT[:, fc, :], in_=ps[:],
                                                     func=AF.Gelu,
                                                     bias=b1_t[:, fc:fc + 1],
                                                     scale=1.0)
                                fc2_mms(fc, (0, 1, 2))
                        finbox["p"] = tc.alloc_tile_pool(name="fin", bufs=1)
                        epilogue(0)
                        epilogue(1)
                        epilogue(2)
                        # pass 2: fc2 for rc3 (w2 already prefetched)
                        psy[3] = ypsum.tile([P, 2, 512], F32, tag="y", name="psy3")
                        for fc in range(FC):
                            fc2_mms(fc, (3,))
                        epilogue(3)
                        finbox["p"].release()
                    w2_pool.release()

            w1_pool.release()


def _row_index(g):
    idx = np.empty(512, dtype=np.int64)
    r = 0
    for p in range(2):
        for s in range(2):
            j = 2 * p + s
            base = j * 512 + g * 128
            idx[r:r + 128] = np.arange(base, base + 128)
            r += 128
    return idx


def _causal_masks(g):
    kj = np.arange(P)[:, None]
    qi = np.arange(P)[None, :]
    m = np.empty((4, P, P), dtype=np.float32)
    for i in range(4):
        m[i] = np.where(kj <= qi + (g - i) * P, 1.0, 0.0)
    return m


def kernel(**inputs):
    if "nc" not in _CACHE:
        _CACHE["nc"] = _build()
    nc = _CACHE["nc"]

    bf = ml_dtypes.bfloat16
    x = np.asarray(inputs["x"], dtype=np.float32)
    w_bf = {k: np.ascontiguousarray(
        np.asarray(inputs[k], dtype=np.float32).astype(bf))
        for k in ("Wq", "Wk", "Wv", "Wo", "W1", "W2")}
    vecs = {k: np.ascontiguousarray(np.asarray(inputs[k], dtype=np.float32))
            for k in ("bq", "bk", "bv", "bo", "b1", "b2", "g1", "be1", "g2",
                      "be2")}

    in_maps = []
    for c in range(N_CORES):
        b, g = c // 4, c % 4
        idx = _row_index(g)
        xb = x[b]
        xrows = xb[idx]
        in_maps.append({
            "xT": np.ascontiguousarray(xb.T.astype(bf)),
            "xrT": np.ascontiguousarray(xrows.T.astype(bf)),
            "xr": np.ascontiguousarray(xrows),
            "wq": w_bf["Wq"], "wk": w_bf["Wk"], "wv": w_bf["Wv"],
            "wo": w_bf["Wo"], "w1": w_bf["W1"], "w2": w_bf["W2"],
            "bq": vecs["bq"], "bk": vecs["bk"],
            "bv": vecs["bv"].astype(bf), "bo": vecs["bo"].astype(bf),
            "b1": vecs["b1"], "b2": vecs["b2"].astype(bf),
            "g1": vecs["g1"].astype(bf), "be1": vecs["be1"].astype(bf),
            "g2": vecs["g2"].astype(bf), "be2": vecs["be2"].astype(bf),
            "cmask": _causal_masks(g).astype(bf),
        })

    res = run_bass_kernel_spmd(nc, in_maps, core_ids=list(range(N_CORES)))
    _CACHE["last_result"] = res

    outp = np.empty((B, L, D), dtype=np.float32)
    for c in range(N_CORES):
        b, g = c // 4, c % 4
        outp[b][_row_index(g)] = res.results[c]["out"].astype(np.float32)
    return outp
